# revision 4
# baseline (speedup 1.0000x reference)
"""Trainium2 Bass kernel for nn_BDLOTreeLSTM_78847009620604.

8-core data parallelism over batch (64 samples/core). Layout: [dim-partitions,
node-cols]; H=256 -> 2 partition halves; 144 (branch, vertex) slots x 64
samples = 9216 node columns per core (parent v -> slot v, c1 v -> 64+v,
c2 v -> 96+v).

- encoder L2 is folded into the pre-gate weights on host:
  pre = h1 @ (enc_w2 @ Wgate) + (enc_b2 @ Wgate + bgate),  h1 = relu(x@w1+b1)
- chain steps: pre-gates are accumulated into PSUM by windowed matmuls (bf16),
  the recurrent h @ U term accumulates on top; gate chunk order [i, u, f, o];
  sigmoid/tanh on ScalarE reading PSUM; cell update on VectorE (+ i*u on
  GPSIMD); h written in bf16 directly into the big state buffers.
- child chains c1/c2 run batched as one stream (slot cols [c1 | c2]).
- biases enter via rank-1 ones-row matmuls (K=1).
"""
import numpy as np
import ml_dtypes

B, H, NP, IN = 512, 256, 64, 9
CS = (32, 48)
COUP = (16, 40)
NB = 3
NCORES = 8
b = B // NCORES                  # 64
NSLOT = NP + CS[0] + CS[1]       # 144
NCOL = NSLOT * b                 # 9216
NG = NCOL // 512                 # 18
WIN = 2                          # chain pre-gate window (steps)

bf16 = ml_dtypes.bfloat16
_cache = {}


def _fuse_weights(g):
    def cat(*xs):
        return np.concatenate(xs, -1)
    Wbu = cat(g['bu_w_iou'][:, :H], g['bu_w_iou'][:, 2*H:3*H], g['bu_w_f'], g['bu_w_iou'][:, H:2*H])
    Ubu = cat(g['bu_u_iou'][:, :H], g['bu_u_iou'][:, 2*H:3*H], g['bu_u_f'], g['bu_u_iou'][:, H:2*H])
    bbu = cat(g['bu_b_iou'][:H], g['bu_b_iou'][2*H:3*H], g['bu_b_f'], g['bu_b_iou'][H:2*H])
    def td_re(w):
        return cat(w[..., :H], w[..., 2*H:3*H], w[..., H:2*H], w[..., 3*H:])
    Wtd = td_re(g['td_w_ih']); Utd = td_re(g['td_w_hh'])
    btd = td_re(g['td_b_ih'] + g['td_b_hh'])
    WpreBU = g['enc_w2'] @ Wbu; bpreBU = g['enc_b2'] @ Wbu + bbu
    WpreTD = g['enc_w2'] @ Wtd; bpreTD = g['enc_b2'] @ Wtd + btd

    def ktile(w):  # [256, C] -> [128, 2, C]
        return np.ascontiguousarray(w.reshape(2, 128, -1).transpose(1, 0, 2))

    return {
        'w1': np.ascontiguousarray(g['enc_w1']).astype(bf16),
        'b1t': np.ascontiguousarray(g['enc_b1'].reshape(2, 128).T).astype(np.float32),
        'WpreBU': ktile(WpreBU).astype(bf16), 'WpreTD': ktile(WpreTD).astype(bf16),
        'UBU': ktile(Ubu).astype(bf16), 'UTD': ktile(Utd).astype(bf16),
        'bpreBU': bpreBU.reshape(1, -1).astype(bf16), 'bpreTD': bpreTD.reshape(1, -1).astype(bf16),
        'dw1': np.ascontiguousarray(g['dec_w1'].reshape(4, 128, H).transpose(1, 0, 2)).astype(bf16),
        'db1': g['dec_b1'].reshape(1, -1).astype(bf16),
        'dw2': ktile(g['dec_w2']).astype(bf16),
        'db2': g['dec_b2'].reshape(1, -1).astype(bf16),
    }


def _pack_x(x_core):
    parts = [x_core[:, 0, :, :].transpose(1, 0, 2).reshape(NP * b, IN),
             x_core[:, 1, :CS[0], :].transpose(1, 0, 2).reshape(CS[0] * b, IN),
             x_core[:, 2, :CS[1], :].transpose(1, 0, 2).reshape(CS[1] * b, IN)]
    xs = np.concatenate(parts, 0)
    return np.ascontiguousarray(xs.T).astype(bf16)


def build_nc():
    import concourse.bass as bass
    import concourse.mybir as mybir
    import concourse.tile as tile
    from concourse import bacc
    from contextlib import ExitStack

    dt = mybir.dt
    AF = mybir.ActivationFunctionType
    AP = bass.AP

    nc = bacc.Bacc(None, target_bir_lowering=False)
    P = {}
    spec = [('xT', (IN, NCOL), dt.bfloat16), ('w1', (IN, H), dt.bfloat16),
            ('b1t', (128, 2), dt.float32),
            ('WpreBU', (128, 2, 4*H), dt.bfloat16), ('WpreTD', (128, 2, 4*H), dt.bfloat16),
            ('UBU', (128, 2, 4*H), dt.bfloat16), ('UTD', (128, 2, 4*H), dt.bfloat16),
            ('bpreBU', (1, 4*H), dt.bfloat16), ('bpreTD', (1, 4*H), dt.bfloat16),
            ('dw1', (128, 4, H), dt.bfloat16), ('db1', (1, H), dt.bfloat16),
            ('dw2', (128, 2, 3), dt.bfloat16), ('db2', (1, 3), dt.bfloat16)]
    for n, shape, d in spec:
        P[n] = nc.declare_dram_parameter(n, list(shape), d, isOutput=False)
    out_d = nc.declare_dram_parameter('out', [3, NCOL], dt.float32, isOutput=True)

    def full(handle, rank):
        return handle[tuple(slice(None) for _ in range(rank))]

    with tile.TileContext(nc) as tc, ExitStack() as ctx:
        sing = ctx.enter_context(tc.tile_pool(name="sing", bufs=1))
        xin = ctx.enter_context(tc.tile_pool(name="xin", bufs=3))
        gpool = ctx.enter_context(tc.tile_pool(name="gpool", bufs=3))
        tpool = ctx.enter_context(tc.tile_pool(name="tpool", bufs=3))
        cpool = ctx.enter_context(tc.tile_pool(name="cpool", bufs=2))
        opool = ctx.enter_context(tc.tile_pool(name="opool", bufs=3))

        W = {}
        for n, shape, d in spec:
            if n == 'xT':
                continue
            t = sing.tile(list(shape), d, tag=f"w_{n}")
            nc.sync.dma_start(out=t, in_=full(P[n], len(shape)))
            W[n] = t
        ones = sing.tile([1, 512], dt.bfloat16, tag="ones")
        nc.vector.memset(ones, 1.0)
        zrow = sing.tile([1, 1024], dt.bfloat16, tag="zrow")
        nc.vector.memset(zrow, 0.0)

        h1 = sing.tile([128, 2, NCOL], dt.bfloat16, tag="h1")
        bu_h = sing.tile([128, 2, NCOL], dt.bfloat16, tag="bu_h")
        td_h = sing.tile([128, 2, NCOL], dt.bfloat16, tag="td_h")
        croots = sing.tile([128, 2, 128], dt.float32, tag="croots")
        tdseed = sing.tile([128, 2, 128], dt.float32, tag="tdseed")

        # ---------------- encoder ----------------
        with tc.tile_pool(name="eps", bufs=2, space="PSUM") as eps:
            for gi in range(NG):
                xg = xin.tile([IN, 512], dt.bfloat16, tag="xg")
                nc.sync.dma_start(out=xg, in_=P['xT'][:, gi*512:(gi+1)*512])
                pe = eps.tile([128, 2, 512], dt.float32, tag="pe")
                for t in range(2):
                    nc.tensor.matmul(pe[:, t, :], W['w1'][:, t*128:(t+1)*128], xg,
                                     start=True, stop=True)
                for t in range(2):
                    nc.scalar.activation(h1[:, t, gi*512:(gi+1)*512], pe[:, t, :],
                                         AF.Relu, bias=W['b1t'][:, t:t+1])

        # ---------------- helpers ----------------
        def hslice(buf, k, grp):
            """[128, 64] AP: buf[:, k, grp*b:(grp+1)*b]"""
            return buf[:, k, grp*b:(grp+1)*b]

        def hpair(buf, k, grp_lo, gstep):
            """[128, 2, 64] AP over groups {grp_lo, grp_lo+gstep}"""
            base = buf[:, k, :]
            return AP(tensor=base.tensor, offset=base.offset + grp_lo * b,
                      ap=[base.ap[0], [gstep * b, 2], [1, b]])

        def store_ap(buf, grp_lo, gstep, nbr):
            """[128, 2, nbr, 64] write AP into both hh halves of buf."""
            return AP(tensor=buf.tensor, offset=buf.offset + grp_lo * b,
                      ap=[buf.ap[0], [NCOL, 2], [gstep * b, nbr] if nbr > 1 else [0, 1],
                          [1, b]])

        def pre_window(ps, Wpre, bpre, rhs_fn, ncolw):
            cpb = 2048 // (ncolw * 4)       # chunks per PSUM bank
            for j in range(8):
                dst = ps[:, j, 0:ncolw]
                for k in range(2):
                    nc.tensor.matmul(dst, Wpre[:, k, j*128:(j+1)*128], rhs_fn(k),
                                     start=(k == 0 and j % cpb == 0), stop=False,
                                     skip_group_check=True)
                nc.tensor.matmul(dst, bpre[:, j*128:(j+1)*128], ones[:, 0:ncolw],
                                 start=False, stop=False, skip_group_check=True)

        def emit_step(ps, off, n, U, h_aps, c_prev, store, extra=None):
            """One LSTM step on psum cols [off, off+n). h_aps None => leaf step.
            c_prev: [128, 2, n] AP or None. store: [128, 2, nbr, 64] AP.
            extra: (eh_aps, ec_ap, fe_pool, pre_rhs) coupling extra child."""
            sl = lambda j: ps[:, j, off:off+n]
            if extra is not None:
                eh_aps = extra[0]
                for j in (0, 1, 2, 3, 6, 7):
                    for k in range(2):
                        nc.tensor.matmul(sl(j), U[:, k, j*128:(j+1)*128], eh_aps[k],
                                         start=False, stop=False, skip_group_check=True)
            if h_aps is not None:
                for j in range(8):
                    for k in range(2):
                        nc.tensor.matmul(sl(j), U[:, k, j*128:(j+1)*128], h_aps[k],
                                         start=False, stop=(k == 1), skip_group_check=True)
            else:
                for j in range(8):
                    nc.tensor.matmul(sl(j), zrow[:, j*128:(j+1)*128], ones[:, 0:n],
                                     start=False, stop=True, skip_group_check=True)
            gates = gpool.tile([128, 8, 128], dt.float32, tag="gates")
            g = lambda j0, j1: gates[:, j0:j1, 0:n]
            pin = lambda j0, j1: ps[:, j0:j1, off:off+n]
            nc.scalar.activation(g(0, 2), pin(0, 2), AF.Sigmoid)
            nc.scalar.activation(g(2, 4), pin(2, 4), AF.Tanh)
            nc.scalar.activation(g(4, 8), pin(4, 8), AF.Sigmoid)
            t1 = tpool.tile([128, 2, 128], dt.float32, tag="t1")
            nc.gpsimd.tensor_mul(t1[:, :, 0:n], g(0, 2), g(2, 4))
            c_new = cpool.tile([128, 2, 128], dt.float32, tag="c")
            cn = c_new[:, :, 0:n]
            if h_aps is None and c_prev is None:
                nc.vector.tensor_copy(cn, t1[:, :, 0:n])
            else:
                nc.vector.tensor_mul(cn, g(4, 6), c_prev)
                nc.vector.tensor_add(cn, cn, t1[:, :, 0:n])
            if extra is not None:
                eh_aps, ec_ap, fe_pool, pre_rhs = extra
                fe = fe_pool.tile([128, 2, b], dt.float32, tag="fe")
                for hh in range(2):
                    jj = 4 + hh
                    for k in range(2):
                        nc.tensor.matmul(fe[:, hh, :], W['WpreBU'][:, k, jj*128:(jj+1)*128],
                                         pre_rhs[k], start=(hh == 0 and k == 0), stop=False,
                                         skip_group_check=True)
                    nc.tensor.matmul(fe[:, hh, :], W['bpreBU'][:, jj*128:(jj+1)*128],
                                     ones[:, 0:b], start=False, stop=False,
                                     skip_group_check=True)
                    for k in range(2):
                        nc.tensor.matmul(fe[:, hh, :], U[:, k, jj*128:(jj+1)*128],
                                         eh_aps[k], start=False,
                                         stop=(hh == 1 and k == 1), skip_group_check=True)
                fes = tpool.tile([128, 2, b], dt.float32, tag="fes")
                nc.scalar.activation(fes, fe, AF.Sigmoid)
                t3 = tpool.tile([128, 2, b], dt.float32, tag="t3")
                nc.vector.tensor_mul(t3, fes, ec_ap)
                nc.vector.tensor_add(cn, cn, t3)
            tct = tpool.tile([128, 2, 128], dt.float32, tag="tc")
            nc.scalar.activation(tct[:, :, 0:n], cn, AF.Tanh)
            nbr = n // b
            nc.vector.tensor_mul(
                store,
                g(6, 8).rearrange("p c (r s) -> p c r s", s=b),
                tct[:, :, 0:n].rearrange("p c (r s) -> p c r s", s=b))
            return cn

        AluOp = mybir.AluOpType

        # ------------- phase A: bu children, v = 47..0, slots ascend v -------------
        with tc.tile_pool(name="pgA", bufs=2, space="PSUM") as pgA:
            c_prev = None
            ps = None
            for s in range(CS[1]):
                v = CS[1] - 1 - s
                if s % WIN == 0:
                    ps = pgA.tile([128, 8, WIN * 128], dt.float32, tag="pg")
                    v_lo = v - (WIN - 1)
                    def rhs(k, v_lo=v_lo):
                        bb = h1[:, k, :]
                        return AP(tensor=bb.tensor, offset=bb.offset + (64 + v_lo) * b,
                                  ap=[bb.ap[0], [b, WIN], [32 * b, 2], [1, b]])
                    pre_window(ps, W['WpreBU'], W['bpreBU'], rhs, WIN * 128)
                slot_i = (WIN - 1) - (s % WIN)          # ascending v in window
                off = slot_i * 128
                if v > CS[0] - 1:
                    # c2 only; c2 occupies cols [64:128) of the slot
                    h_aps = None if s == 0 else tuple(hslice(bu_h, k, 96 + v + 1) for k in range(2))
                    store = store_ap(bu_h, 96 + v, 0, 1)
                    c_prev = emit_step(ps, off + 64, 64, W['UBU'], h_aps, c_prev, store)
                elif v == CS[0] - 1:
                    # c1 leaf joins: c1 half (cols 0:64) first-step, c2 half normal
                    h_aps = tuple(hslice(bu_h, k, 96 + v + 1) for k in range(2))
                    sl = lambda j: ps[:, j, off+64:off+128]
                    for j in range(8):
                        for k in range(2):
                            nc.tensor.matmul(sl(j), W['UBU'][:, k, j*128:(j+1)*128],
                                             h_aps[k], start=False, stop=(k == 1),
                                             skip_group_check=True)
                    gates = gpool.tile([128, 8, 128], dt.float32, tag="gates")
                    pin = lambda j0, j1: ps[:, j0:j1, off:off+128]
                    nc.scalar.activation(gates[:, 0:2, :], pin(0, 2), AF.Sigmoid)
                    nc.scalar.activation(gates[:, 2:4, :], pin(2, 4), AF.Tanh)
                    nc.scalar.activation(gates[:, 4:8, :], pin(4, 8), AF.Sigmoid)
                    t1 = tpool.tile([128, 2, 128], dt.float32, tag="t1")
                    nc.gpsimd.tensor_mul(t1, gates[:, 0:2, :], gates[:, 2:4, :])
                    c_new = cpool.tile([128, 2, 128], dt.float32, tag="c")
                    nc.vector.tensor_copy(c_new[:, :, 0:64], t1[:, :, 0:64])
                    nc.vector.tensor_mul(c_new[:, :, 64:128], gates[:, 4:6, 64:128], c_prev)
                    nc.vector.tensor_add(c_new[:, :, 64:128], c_new[:, :, 64:128],
                                         t1[:, :, 64:128])
                    tct = tpool.tile([128, 2, 128], dt.float32, tag="tc")
                    nc.scalar.activation(tct, c_new, AF.Tanh)
                    store = store_ap(bu_h, 64 + v, 32, 2)
                    nc.vector.tensor_mul(
                        store,
                        gates[:, 6:8, :].rearrange("p c (r s) -> p c r s", s=b),
                        tct.rearrange("p c (r s) -> p c r s", s=b))
                    c_prev = c_new
                else:
                    h_aps = tuple(hpair(bu_h, k, 64 + v + 1, 32) for k in range(2))
                    store = store_ap(bu_h, 64 + v, 32, 2)
                    c_prev = emit_step(ps, off, 128, W['UBU'], h_aps, c_prev, store)
            nc.vector.tensor_copy(croots, c_prev)

        # ------------- phase B: bu parent, v = 63..0 -------------
        with tc.tile_pool(name="pgB", bufs=2, space="PSUM") as pgB, \
             tc.tile_pool(name="feB", bufs=2, space="PSUM") as feB:
            c_prev = None
            ps = None
            for s in range(NP):
                v = NP - 1 - s
                if s % WIN == 0:
                    ps = pgB.tile([128, 8, WIN * 64], dt.float32, tag="pg")
                    v_lo = v - (WIN - 1)
                    def rhs(k, v_lo=v_lo):
                        bb = h1[:, k, :]
                        return AP(tensor=bb.tensor, offset=bb.offset + v_lo * b,
                                  ap=[bb.ap[0], [b, WIN], [1, b]])
                    pre_window(ps, W['WpreBU'], W['bpreBU'], rhs, WIN * 64)
                off = ((WIN - 1) - (s % WIN)) * 64
                extra = None
                if v == COUP[0]:
                    eh = tuple(hslice(bu_h, k, 64) for k in range(2))
                    extra = (eh, croots[:, :, 0:64], feB,
                             tuple(hslice(h1, k, v) for k in range(2)))
                elif v == COUP[1]:
                    eh = tuple(hslice(bu_h, k, 96) for k in range(2))
                    extra = (eh, croots[:, :, 64:128], feB,
                             tuple(hslice(h1, k, v) for k in range(2)))
                h_aps = None if s == 0 else tuple(hslice(bu_h, k, v + 1) for k in range(2))
                store = store_ap(bu_h, v, 0, 1)
                c_prev = emit_step(ps, off, 64, W['UBU'], h_aps, c_prev, store, extra=extra)

        nc.vector.tensor_copy(td_h[:, :, 0:b], bu_h[:, :, 0:b])

        # ------------- phase C: td parent, v = 1..63 -------------
        with tc.tile_pool(name="pgC", bufs=2, space="PSUM") as pgC:
            ps = None
            for i, v in enumerate(range(1, NP)):
                if i % WIN == 0:
                    ps = pgC.tile([128, 8, WIN * 64], dt.float32, tag="pg")
                    def rhs(k, v0=v):
                        bb = h1[:, k, :]
                        return AP(tensor=bb.tensor, offset=bb.offset + v0 * b,
                                  ap=[bb.ap[0], [b, WIN], [1, b]])
                    pre_window(ps, W['WpreTD'], W['bpreTD'], rhs, WIN * 64)
                off = (i % WIN) * 64
                h_aps = tuple(hslice(td_h, k, v - 1) for k in range(2))
                store = store_ap(td_h, v, 0, 1)
                c_prev = emit_step(ps, off, 64, W['UTD'], h_aps, c_prev, store)
                if v == COUP[0]:
                    nc.vector.tensor_copy(tdseed[:, :, 0:64], c_prev)
                elif v == COUP[1]:
                    nc.vector.tensor_copy(tdseed[:, :, 64:128], c_prev)

        # ------------- phase D: td children, v = 0..47 -------------
        with tc.tile_pool(name="pgD", bufs=2, space="PSUM") as pgD:
            c_prev = None
            ps = None
            for s in range(CS[1]):
                v = s
                if s % WIN == 0:
                    ps = pgD.tile([128, 8, WIN * 128], dt.float32, tag="pg")
                    def rhs(k, v0=v):
                        bb = h1[:, k, :]
                        return AP(tensor=bb.tensor, offset=bb.offset + (64 + v0) * b,
                                  ap=[bb.ap[0], [b, WIN], [32 * b, 2], [1, b]])
                    pre_window(ps, W['WpreTD'], W['bpreTD'], rhs, WIN * 128)
                off = (s % WIN) * 128
                if s == 0:
                    h_aps = tuple(
                        AP(tensor=td_h[:, k, :].tensor,
                           offset=td_h[:, k, :].offset + COUP[0] * b,
                           ap=[td_h[:, k, :].ap[0], [(COUP[1] - COUP[0]) * b, 2], [1, b]])
                        for k in range(2))
                    store = store_ap(td_h, 64 + v, 32, 2)
                    c_prev = emit_step(ps, off, 128, W['UTD'], h_aps, tdseed, store)
                elif v < CS[0]:
                    h_aps = tuple(hpair(td_h, k, 64 + v - 1, 32) for k in range(2))
                    store = store_ap(td_h, 64 + v, 32, 2)
                    c_prev = emit_step(ps, off, 128, W['UTD'], h_aps, c_prev, store)
                else:
                    h_aps = tuple(hslice(td_h, k, 96 + v - 1) for k in range(2))
                    store = store_ap(td_h, 96 + v, 0, 1)
                    cp = c_prev[:, :, 64:128] if v == CS[0] else c_prev
                    c_prev = emit_step(ps, off + 64, 64, W['UTD'], h_aps, cp, store)

        # ------------- decoder -------------
        with tc.tile_pool(name="dps", bufs=2, space="PSUM") as dps, \
             tc.tile_pool(name="ops", bufs=2, space="PSUM") as ops:
            for gi in range(NG):
                cs0, cs1 = gi * 512, (gi + 1) * 512
                pd = dps.tile([128, 2, 512], dt.float32, tag="pd")
                for m in range(2):
                    for k in range(4):
                        srcbuf = bu_h if k < 2 else td_h
                        nc.tensor.matmul(pd[:, m, :], W['dw1'][:, k, m*128:(m+1)*128],
                                         srcbuf[:, k % 2, cs0:cs1],
                                         start=(k == 0), stop=False)
                    nc.tensor.matmul(pd[:, m, :], W['db1'][:, m*128:(m+1)*128],
                                     ones[:, 0:512], start=False, stop=True)
                h2 = opool.tile([128, 2, 512], dt.bfloat16, tag="h2")
                for m in range(2):
                    nc.scalar.activation(h2[:, m, :], pd[:, m, :], AF.Relu)
                po = ops.tile([3, 512], dt.float32, tag="po")
                for k in range(2):
                    nc.tensor.matmul(po, W['dw2'][:, k, :], h2[:, k, :],
                                     start=(k == 0), stop=False)
                nc.tensor.matmul(po, W['db2'], ones[:, 0:512], start=False, stop=True)
                og = opool.tile([3, 512], dt.float32, tag="og")
                nc.vector.tensor_copy(og, po)
                nc.sync.dma_start(out=out_d[:, cs0:cs1], in_=og)

    nc.finalize()
    return nc


def kernel(**inputs):
    from concourse.bass_utils import run_bass_kernel_spmd

    g = {k: np.asarray(v, np.float32) for k, v in inputs.items()}
    x = g['x']
    Wd = _fuse_weights(g)

    if 'nc' not in _cache:
        _cache['nc'] = build_nc()
    nc = _cache['nc']

    in_maps = []
    for c in range(NCORES):
        m = dict(Wd)
        m['xT'] = _pack_x(x[c*b:(c+1)*b])
        in_maps.append(m)
    res = run_bass_kernel_spmd(nc, in_maps, list(range(NCORES)))

    const = (np.maximum(g['dec_b1'], 0) @ g['dec_w2'] + g['dec_b2']).astype(np.float32)
    out = np.empty((B, NB, NP, 3), np.float32)
    out[:] = const
    for c in range(NCORES):
        o = np.asarray(res.results[c]['out'])      # [3, 9216]
        sl = o.T.reshape(NSLOT, b, 3).transpose(1, 0, 2)  # [sample, slot, 3]
        out[c*b:(c+1)*b, 0, :, :] = sl[:, :NP]
        out[c*b:(c+1)*b, 1, :CS[0], :] = sl[:, NP:NP+CS[0]]
        out[c*b:(c+1)*b, 2, :CS[1], :] = sl[:, NP+CS[0]:]
    return out


# revision 5
# speedup vs baseline: 1.0459x; 1.0459x over previous
"""Trainium2 Bass kernel for nn_BDLOTreeLSTM_78847009620604.

8-core data parallelism over batch (64 samples/core). Layout: [dim-partitions,
node-cols]; H=256 -> 2 partition halves; 144 (branch, vertex) slots x 64
samples = 9216 node columns per core (parent v -> slot v, c1 v -> 64+v,
c2 v -> 96+v).

- encoder L2 is folded into the pre-gate weights on host:
  pre = h1 @ (enc_w2 @ Wgate) + (enc_b2 @ Wgate + bgate),  h1 = relu(x@w1+b1)
- chain steps: pre-gates are accumulated into PSUM by windowed matmuls (bf16),
  the recurrent h @ U term accumulates on top; gate chunk order [i, u, f, o];
  sigmoid/tanh on ScalarE reading PSUM; cell update on VectorE (+ i*u on
  GPSIMD); h written in bf16 directly into the big state buffers.
- child chains c1/c2 run batched as one stream (slot cols [c1 | c2]).
- biases enter via rank-1 ones-row matmuls (K=1).
"""
import numpy as np
import ml_dtypes

B, H, NP, IN = 512, 256, 64, 9
CS = (32, 48)
COUP = (16, 40)
NB = 3
NCORES = 8
b = B // NCORES                  # 64
NSLOT = NP + CS[0] + CS[1]       # 144
NCOL = NSLOT * b                 # 9216
NG = NCOL // 512                 # 18
WIN = 2                          # chain pre-gate window (steps)

bf16 = ml_dtypes.bfloat16
_cache = {}


def _fuse_weights(g):
    def cat(*xs):
        return np.concatenate(xs, -1)
    Wbu = cat(g['bu_w_iou'][:, 2*H:3*H], g['bu_w_iou'][:, :H], g['bu_w_f'], g['bu_w_iou'][:, H:2*H])
    Ubu = cat(g['bu_u_iou'][:, 2*H:3*H], g['bu_u_iou'][:, :H], g['bu_u_f'], g['bu_u_iou'][:, H:2*H])
    bbu = cat(g['bu_b_iou'][2*H:3*H], g['bu_b_iou'][:H], g['bu_b_f'], g['bu_b_iou'][H:2*H])
    def td_re(w):
        return cat(w[..., 2*H:3*H], w[..., :H], w[..., H:2*H], w[..., 3*H:])
    Wtd = td_re(g['td_w_ih']); Utd = td_re(g['td_w_hh'])
    btd = td_re(g['td_b_ih'] + g['td_b_hh'])
    WpreBU = g['enc_w2'] @ Wbu; bpreBU = g['enc_b2'] @ Wbu + bbu
    WpreTD = g['enc_w2'] @ Wtd; bpreTD = g['enc_b2'] @ Wtd + btd

    def ktile(w):  # [256, C] -> [128, 2, C]
        return np.ascontiguousarray(w.reshape(2, 128, -1).transpose(1, 0, 2))

    return {
        'w1': np.ascontiguousarray(g['enc_w1']).astype(bf16),
        'b1t': np.ascontiguousarray(g['enc_b1'].reshape(2, 128).T).astype(np.float32),
        'WpreBU': ktile(WpreBU).astype(bf16), 'WpreTD': ktile(WpreTD).astype(bf16),
        'UBU': ktile(Ubu).astype(bf16), 'UTD': ktile(Utd).astype(bf16),
        'bpreBU': bpreBU.reshape(1, -1).astype(bf16), 'bpreTD': bpreTD.reshape(1, -1).astype(bf16),
        'dw1': np.ascontiguousarray(g['dec_w1'].reshape(4, 128, H).transpose(1, 0, 2)).astype(bf16),
        'db1': g['dec_b1'].reshape(1, -1).astype(bf16),
        'dw2': ktile(g['dec_w2']).astype(bf16),
        'db2': g['dec_b2'].reshape(1, -1).astype(bf16),
    }


def _pack_x(x_core):
    parts = [x_core[:, 0, :, :].transpose(1, 0, 2).reshape(NP * b, IN),
             x_core[:, 1, :CS[0], :].transpose(1, 0, 2).reshape(CS[0] * b, IN),
             x_core[:, 2, :CS[1], :].transpose(1, 0, 2).reshape(CS[1] * b, IN)]
    xs = np.concatenate(parts, 0)
    return np.ascontiguousarray(xs.T).astype(bf16)


def build_nc():
    import concourse.bass as bass
    import concourse.mybir as mybir
    import concourse.tile as tile
    from concourse import bacc
    from contextlib import ExitStack

    dt = mybir.dt
    AF = mybir.ActivationFunctionType
    AP = bass.AP

    nc = bacc.Bacc(None, target_bir_lowering=False)
    P = {}
    spec = [('xT', (IN, NCOL), dt.bfloat16), ('w1', (IN, H), dt.bfloat16),
            ('b1t', (128, 2), dt.float32),
            ('WpreBU', (128, 2, 4*H), dt.bfloat16), ('WpreTD', (128, 2, 4*H), dt.bfloat16),
            ('UBU', (128, 2, 4*H), dt.bfloat16), ('UTD', (128, 2, 4*H), dt.bfloat16),
            ('bpreBU', (1, 4*H), dt.bfloat16), ('bpreTD', (1, 4*H), dt.bfloat16),
            ('dw1', (128, 4, H), dt.bfloat16), ('db1', (1, H), dt.bfloat16),
            ('dw2', (128, 2, 3), dt.bfloat16), ('db2', (1, 3), dt.bfloat16)]
    for n, shape, d in spec:
        P[n] = nc.declare_dram_parameter(n, list(shape), d, isOutput=False)
    out_d = nc.declare_dram_parameter('out', [3, NCOL], dt.float32, isOutput=True)

    def full(handle, rank):
        return handle[tuple(slice(None) for _ in range(rank))]

    with tile.TileContext(nc) as tc, ExitStack() as ctx:
        sing = ctx.enter_context(tc.tile_pool(name="sing", bufs=1))
        xin = ctx.enter_context(tc.tile_pool(name="xin", bufs=3))
        gpool = ctx.enter_context(tc.tile_pool(name="gpool", bufs=3))
        tpool = ctx.enter_context(tc.tile_pool(name="tpool", bufs=3))
        cpool = ctx.enter_context(tc.tile_pool(name="cpool", bufs=2))
        opool = ctx.enter_context(tc.tile_pool(name="opool", bufs=3))

        W = {}
        for n, shape, d in spec:
            if n == 'xT':
                continue
            t = sing.tile(list(shape), d, tag=f"w_{n}")
            nc.sync.dma_start(out=t, in_=full(P[n], len(shape)))
            W[n] = t
        ones = sing.tile([1, 512], dt.bfloat16, tag="ones")
        nc.vector.memset(ones, 1.0)
        zrow = sing.tile([1, 1024], dt.bfloat16, tag="zrow")
        nc.vector.memset(zrow, 0.0)

        h1 = sing.tile([128, 2, NCOL], dt.bfloat16, tag="h1")
        bu_h = sing.tile([128, 2, NCOL], dt.bfloat16, tag="bu_h")
        td_h = sing.tile([128, 2, NCOL], dt.bfloat16, tag="td_h")
        croots = sing.tile([128, 2, 128], dt.float32, tag="croots")
        tdseed = sing.tile([128, 2, 128], dt.float32, tag="tdseed")

        # ---------------- encoder ----------------
        with tc.tile_pool(name="eps", bufs=2, space="PSUM") as eps:
            for gi in range(NG):
                xg = xin.tile([IN, 512], dt.bfloat16, tag="xg")
                nc.sync.dma_start(out=xg, in_=P['xT'][:, gi*512:(gi+1)*512])
                pe = eps.tile([128, 2, 512], dt.float32, tag="pe")
                for t in range(2):
                    nc.tensor.matmul(pe[:, t, :], W['w1'][:, t*128:(t+1)*128], xg,
                                     start=True, stop=True)
                for t in range(2):
                    nc.scalar.activation(h1[:, t, gi*512:(gi+1)*512], pe[:, t, :],
                                         AF.Relu, bias=W['b1t'][:, t:t+1])

        # ---------------- helpers ----------------
        def hslice(buf, k, grp):
            """[128, 64] AP: buf[:, k, grp*b:(grp+1)*b]"""
            return buf[:, k, grp*b:(grp+1)*b]

        def hpair(buf, k, grp_lo, gstep):
            """[128, 2, 64] AP over groups {grp_lo, grp_lo+gstep}"""
            base = buf[:, k, :]
            return AP(tensor=base.tensor, offset=base.offset + grp_lo * b,
                      ap=[base.ap[0], [gstep * b, 2], [1, b]])

        def store_ap(buf, grp_lo, gstep, nbr):
            """[128, 2, nbr, 64] write AP into both hh halves of buf."""
            return AP(tensor=buf.tensor, offset=buf.offset + grp_lo * b,
                      ap=[buf.ap[0], [NCOL, 2], [gstep * b, nbr] if nbr > 1 else [0, 1],
                          [1, b]])

        def pre_window(ps, Wpre, bpre, rhs_fn, ncolw):
            cpb = 2048 // (ncolw * 4)       # chunks per PSUM bank
            for j in range(8):
                dst = ps[:, j, 0:ncolw]
                for k in range(2):
                    nc.tensor.matmul(dst, Wpre[:, k, j*128:(j+1)*128], rhs_fn(k),
                                     start=(k == 0 and j % cpb == 0), stop=False,
                                     skip_group_check=True)
                nc.tensor.matmul(dst, bpre[:, j*128:(j+1)*128], ones[:, 0:ncolw],
                                 start=False, stop=False, skip_group_check=True)

        def emit_step(ps, off, n, U, h_aps, c_prev, store, extra=None):
            """One LSTM step on psum cols [off, off+n). h_aps None => leaf step.
            c_prev: [128, 2, n] AP or None. store: [128, 2, nbr, 64] AP.
            extra: (eh_aps, ec_ap, fe_pool, pre_rhs) coupling extra child."""
            sl = lambda j: ps[:, j, off:off+n]
            if extra is not None:
                eh_aps = extra[0]
                for j in (0, 1, 2, 3, 6, 7):
                    for k in range(2):
                        nc.tensor.matmul(sl(j), U[:, k, j*128:(j+1)*128], eh_aps[k],
                                         start=False, stop=False, skip_group_check=True)
            if h_aps is not None:
                for j in range(8):
                    for k in range(2):
                        nc.tensor.matmul(sl(j), U[:, k, j*128:(j+1)*128], h_aps[k],
                                         start=False, stop=(k == 1), skip_group_check=True)
            else:
                for j in range(8):
                    nc.tensor.matmul(sl(j), zrow[:, j*128:(j+1)*128], ones[:, 0:n],
                                     start=False, stop=True, skip_group_check=True)
            gates = gpool.tile([128, 8, 128], dt.float32, tag="gates")
            g = lambda j0, j1: gates[:, j0:j1, 0:n]
            pin = lambda j0, j1: ps[:, j0:j1, off:off+n]
            nc.scalar.activation(g(0, 2), pin(0, 2), AF.Tanh)
            nc.scalar.activation(g(2, 8), pin(2, 8), AF.Sigmoid)
            t1 = tpool.tile([128, 2, 128], dt.float32, tag="t1")
            nc.gpsimd.tensor_mul(t1[:, :, 0:n], g(2, 4), g(0, 2))
            c_new = cpool.tile([128, 2, 128], dt.float32, tag="c")
            cn = c_new[:, :, 0:n]
            if h_aps is None and c_prev is None:
                nc.vector.tensor_copy(cn, t1[:, :, 0:n])
            else:
                nc.vector.tensor_mul(cn, g(4, 6), c_prev)
                nc.vector.tensor_add(cn, cn, t1[:, :, 0:n])
            if extra is not None:
                eh_aps, ec_ap, fe_pool, pre_rhs = extra
                fe = fe_pool.tile([128, 2, b], dt.float32, tag="fe")
                for hh in range(2):
                    jj = 4 + hh
                    for k in range(2):
                        nc.tensor.matmul(fe[:, hh, :], W['WpreBU'][:, k, jj*128:(jj+1)*128],
                                         pre_rhs[k], start=(hh == 0 and k == 0), stop=False,
                                         skip_group_check=True)
                    nc.tensor.matmul(fe[:, hh, :], W['bpreBU'][:, jj*128:(jj+1)*128],
                                     ones[:, 0:b], start=False, stop=False,
                                     skip_group_check=True)
                    for k in range(2):
                        nc.tensor.matmul(fe[:, hh, :], U[:, k, jj*128:(jj+1)*128],
                                         eh_aps[k], start=False,
                                         stop=(hh == 1 and k == 1), skip_group_check=True)
                fes = tpool.tile([128, 2, b], dt.float32, tag="fes")
                nc.scalar.activation(fes, fe, AF.Sigmoid)
                t3 = tpool.tile([128, 2, b], dt.float32, tag="t3")
                nc.vector.tensor_mul(t3, fes, ec_ap)
                nc.vector.tensor_add(cn, cn, t3)
            tct = tpool.tile([128, 2, 128], dt.float32, tag="tc")
            nc.scalar.activation(tct[:, :, 0:n], cn, AF.Tanh)
            nbr = n // b
            nc.vector.tensor_mul(
                store,
                g(6, 8).rearrange("p c (r s) -> p c r s", s=b),
                tct[:, :, 0:n].rearrange("p c (r s) -> p c r s", s=b))
            return cn

        AluOp = mybir.AluOpType

        # ------------- phase A: bu children, v = 47..0, slots ascend v -------------
        with tc.tile_pool(name="pgA", bufs=2, space="PSUM") as pgA:
            c_prev = None
            ps = None
            for s in range(CS[1]):
                v = CS[1] - 1 - s
                if s % WIN == 0:
                    ps = pgA.tile([128, 8, WIN * 128], dt.float32, tag="pg")
                    v_lo = v - (WIN - 1)
                    def rhs(k, v_lo=v_lo):
                        bb = h1[:, k, :]
                        return AP(tensor=bb.tensor, offset=bb.offset + (64 + v_lo) * b,
                                  ap=[bb.ap[0], [b, WIN], [32 * b, 2], [1, b]])
                    pre_window(ps, W['WpreBU'], W['bpreBU'], rhs, WIN * 128)
                slot_i = (WIN - 1) - (s % WIN)          # ascending v in window
                off = slot_i * 128
                if v > CS[0] - 1:
                    # c2 only; c2 occupies cols [64:128) of the slot
                    h_aps = None if s == 0 else tuple(hslice(bu_h, k, 96 + v + 1) for k in range(2))
                    store = store_ap(bu_h, 96 + v, 0, 1)
                    c_prev = emit_step(ps, off + 64, 64, W['UBU'], h_aps, c_prev, store)
                elif v == CS[0] - 1:
                    # c1 leaf joins: c1 half (cols 0:64) first-step, c2 half normal
                    h_aps = tuple(hslice(bu_h, k, 96 + v + 1) for k in range(2))
                    sl = lambda j: ps[:, j, off+64:off+128]
                    for j in range(8):
                        for k in range(2):
                            nc.tensor.matmul(sl(j), W['UBU'][:, k, j*128:(j+1)*128],
                                             h_aps[k], start=False, stop=(k == 1),
                                             skip_group_check=True)
                    gates = gpool.tile([128, 8, 128], dt.float32, tag="gates")
                    pin = lambda j0, j1: ps[:, j0:j1, off:off+128]
                    nc.scalar.activation(gates[:, 0:2, :], pin(0, 2), AF.Tanh)
                    nc.scalar.activation(gates[:, 2:8, :], pin(2, 8), AF.Sigmoid)
                    t1 = tpool.tile([128, 2, 128], dt.float32, tag="t1")
                    nc.gpsimd.tensor_mul(t1, gates[:, 2:4, :], gates[:, 0:2, :])
                    c_new = cpool.tile([128, 2, 128], dt.float32, tag="c")
                    nc.vector.tensor_copy(c_new[:, :, 0:64], t1[:, :, 0:64])
                    nc.vector.tensor_mul(c_new[:, :, 64:128], gates[:, 4:6, 64:128], c_prev)
                    nc.vector.tensor_add(c_new[:, :, 64:128], c_new[:, :, 64:128],
                                         t1[:, :, 64:128])
                    tct = tpool.tile([128, 2, 128], dt.float32, tag="tc")
                    nc.scalar.activation(tct, c_new, AF.Tanh)
                    store = store_ap(bu_h, 64 + v, 32, 2)
                    nc.vector.tensor_mul(
                        store,
                        gates[:, 6:8, :].rearrange("p c (r s) -> p c r s", s=b),
                        tct.rearrange("p c (r s) -> p c r s", s=b))
                    c_prev = c_new
                else:
                    h_aps = tuple(hpair(bu_h, k, 64 + v + 1, 32) for k in range(2))
                    store = store_ap(bu_h, 64 + v, 32, 2)
                    c_prev = emit_step(ps, off, 128, W['UBU'], h_aps, c_prev, store)
            nc.vector.tensor_copy(croots, c_prev)

        # ------------- phase B: bu parent, v = 63..0 -------------
        with tc.tile_pool(name="pgB", bufs=2, space="PSUM") as pgB, \
             tc.tile_pool(name="feB", bufs=2, space="PSUM") as feB:
            c_prev = None
            ps = None
            for s in range(NP):
                v = NP - 1 - s
                if s % WIN == 0:
                    ps = pgB.tile([128, 8, WIN * 64], dt.float32, tag="pg")
                    v_lo = v - (WIN - 1)
                    def rhs(k, v_lo=v_lo):
                        bb = h1[:, k, :]
                        return AP(tensor=bb.tensor, offset=bb.offset + v_lo * b,
                                  ap=[bb.ap[0], [b, WIN], [1, b]])
                    pre_window(ps, W['WpreBU'], W['bpreBU'], rhs, WIN * 64)
                off = ((WIN - 1) - (s % WIN)) * 64
                extra = None
                if v == COUP[0]:
                    eh = tuple(hslice(bu_h, k, 64) for k in range(2))
                    extra = (eh, croots[:, :, 0:64], feB,
                             tuple(hslice(h1, k, v) for k in range(2)))
                elif v == COUP[1]:
                    eh = tuple(hslice(bu_h, k, 96) for k in range(2))
                    extra = (eh, croots[:, :, 64:128], feB,
                             tuple(hslice(h1, k, v) for k in range(2)))
                h_aps = None if s == 0 else tuple(hslice(bu_h, k, v + 1) for k in range(2))
                store = store_ap(bu_h, v, 0, 1)
                c_prev = emit_step(ps, off, 64, W['UBU'], h_aps, c_prev, store, extra=extra)

        nc.vector.tensor_copy(td_h[:, :, 0:b], bu_h[:, :, 0:b])

        # ------------- phase C: td parent, v = 1..63 -------------
        with tc.tile_pool(name="pgC", bufs=2, space="PSUM") as pgC:
            ps = None
            for i, v in enumerate(range(1, NP)):
                if i % WIN == 0:
                    ps = pgC.tile([128, 8, WIN * 64], dt.float32, tag="pg")
                    def rhs(k, v0=v):
                        bb = h1[:, k, :]
                        return AP(tensor=bb.tensor, offset=bb.offset + v0 * b,
                                  ap=[bb.ap[0], [b, WIN], [1, b]])
                    pre_window(ps, W['WpreTD'], W['bpreTD'], rhs, WIN * 64)
                off = (i % WIN) * 64
                h_aps = tuple(hslice(td_h, k, v - 1) for k in range(2))
                store = store_ap(td_h, v, 0, 1)
                c_prev = emit_step(ps, off, 64, W['UTD'], h_aps, c_prev, store)
                if v == COUP[0]:
                    nc.vector.tensor_copy(tdseed[:, :, 0:64], c_prev)
                elif v == COUP[1]:
                    nc.vector.tensor_copy(tdseed[:, :, 64:128], c_prev)

        # ------------- phase D: td children, v = 0..47 -------------
        with tc.tile_pool(name="pgD", bufs=2, space="PSUM") as pgD:
            c_prev = None
            ps = None
            for s in range(CS[1]):
                v = s
                if s % WIN == 0:
                    ps = pgD.tile([128, 8, WIN * 128], dt.float32, tag="pg")
                    def rhs(k, v0=v):
                        bb = h1[:, k, :]
                        return AP(tensor=bb.tensor, offset=bb.offset + (64 + v0) * b,
                                  ap=[bb.ap[0], [b, WIN], [32 * b, 2], [1, b]])
                    pre_window(ps, W['WpreTD'], W['bpreTD'], rhs, WIN * 128)
                off = (s % WIN) * 128
                if s == 0:
                    h_aps = tuple(
                        AP(tensor=td_h[:, k, :].tensor,
                           offset=td_h[:, k, :].offset + COUP[0] * b,
                           ap=[td_h[:, k, :].ap[0], [(COUP[1] - COUP[0]) * b, 2], [1, b]])
                        for k in range(2))
                    store = store_ap(td_h, 64 + v, 32, 2)
                    c_prev = emit_step(ps, off, 128, W['UTD'], h_aps, tdseed, store)
                elif v < CS[0]:
                    h_aps = tuple(hpair(td_h, k, 64 + v - 1, 32) for k in range(2))
                    store = store_ap(td_h, 64 + v, 32, 2)
                    c_prev = emit_step(ps, off, 128, W['UTD'], h_aps, c_prev, store)
                else:
                    h_aps = tuple(hslice(td_h, k, 96 + v - 1) for k in range(2))
                    store = store_ap(td_h, 96 + v, 0, 1)
                    cp = c_prev[:, :, 64:128] if v == CS[0] else c_prev
                    c_prev = emit_step(ps, off + 64, 64, W['UTD'], h_aps, cp, store)

        # ------------- decoder -------------
        with tc.tile_pool(name="dps", bufs=2, space="PSUM") as dps, \
             tc.tile_pool(name="ops", bufs=2, space="PSUM") as ops:
            for gi in range(NG):
                cs0, cs1 = gi * 512, (gi + 1) * 512
                pd = dps.tile([128, 2, 512], dt.float32, tag="pd")
                for m in range(2):
                    for k in range(4):
                        srcbuf = bu_h if k < 2 else td_h
                        nc.tensor.matmul(pd[:, m, :], W['dw1'][:, k, m*128:(m+1)*128],
                                         srcbuf[:, k % 2, cs0:cs1],
                                         start=(k == 0), stop=False)
                    nc.tensor.matmul(pd[:, m, :], W['db1'][:, m*128:(m+1)*128],
                                     ones[:, 0:512], start=False, stop=True)
                h2 = opool.tile([128, 2, 512], dt.bfloat16, tag="h2")
                for m in range(2):
                    nc.scalar.activation(h2[:, m, :], pd[:, m, :], AF.Relu)
                po = ops.tile([3, 512], dt.float32, tag="po")
                for k in range(2):
                    nc.tensor.matmul(po, W['dw2'][:, k, :], h2[:, k, :],
                                     start=(k == 0), stop=False)
                nc.tensor.matmul(po, W['db2'], ones[:, 0:512], start=False, stop=True)
                og = opool.tile([3, 512], dt.float32, tag="og")
                nc.vector.tensor_copy(og, po)
                nc.sync.dma_start(out=out_d[:, cs0:cs1], in_=og)

    nc.finalize()
    return nc


def kernel(**inputs):
    from concourse.bass_utils import run_bass_kernel_spmd

    g = {k: np.asarray(v, np.float32) for k, v in inputs.items()}
    x = g['x']
    Wd = _fuse_weights(g)

    if 'nc' not in _cache:
        _cache['nc'] = build_nc()
    nc = _cache['nc']

    in_maps = []
    for c in range(NCORES):
        m = dict(Wd)
        m['xT'] = _pack_x(x[c*b:(c+1)*b])
        in_maps.append(m)
    res = run_bass_kernel_spmd(nc, in_maps, list(range(NCORES)))

    const = (np.maximum(g['dec_b1'], 0) @ g['dec_w2'] + g['dec_b2']).astype(np.float32)
    out = np.empty((B, NB, NP, 3), np.float32)
    out[:] = const
    for c in range(NCORES):
        o = np.asarray(res.results[c]['out'])      # [3, 9216]
        sl = o.T.reshape(NSLOT, b, 3).transpose(1, 0, 2)  # [sample, slot, 3]
        out[c*b:(c+1)*b, 0, :, :] = sl[:, :NP]
        out[c*b:(c+1)*b, 1, :CS[0], :] = sl[:, NP:NP+CS[0]]
        out[c*b:(c+1)*b, 2, :CS[1], :] = sl[:, NP+CS[0]:]
    return out


# revision 6
# speedup vs baseline: 1.1061x; 1.0575x over previous
"""Trainium2 Bass kernel for nn_BDLOTreeLSTM_78847009620604.

8-core data parallelism over batch (64 samples/core). Layout: [dim-partitions,
node-cols]; H=256 -> 2 partition halves; 144 (branch, vertex) slots x 64
samples = 9216 node columns per core (parent v -> slot v, c1 v -> 64+v,
c2 v -> 96+v).

- encoder L2 is folded into the pre-gate weights on host:
  pre = h1 @ (enc_w2 @ Wgate) + (enc_b2 @ Wgate + bgate),  h1 = relu(x@w1+b1)
- chain steps: pre-gates are accumulated into PSUM by windowed matmuls (bf16),
  the recurrent h @ U term accumulates on top; gate chunk order [i, u, f, o];
  sigmoid/tanh on ScalarE reading PSUM; cell update on VectorE (+ i*u on
  GPSIMD); h written in bf16 directly into the big state buffers.
- child chains c1/c2 run batched as one stream (slot cols [c1 | c2]).
- biases enter via rank-1 ones-row matmuls (K=1).
"""
import numpy as np
import ml_dtypes

B, H, NP, IN = 512, 256, 64, 9
CS = (32, 48)
COUP = (16, 40)
NB = 3
NCORES = 8
b = B // NCORES                  # 64
NSLOT = NP + CS[0] + CS[1]       # 144
NCOL = NSLOT * b                 # 9216
NG = NCOL // 512                 # 18
WIN = 2                          # chain pre-gate window (steps)

bf16 = ml_dtypes.bfloat16
_cache = {}


def _fuse_weights(g):
    def cat(*xs):
        return np.concatenate(xs, -1)
    Wbu = cat(g['bu_w_iou'][:, 2*H:3*H], g['bu_w_iou'][:, :H], g['bu_w_f'], g['bu_w_iou'][:, H:2*H])
    Ubu = cat(g['bu_u_iou'][:, 2*H:3*H], g['bu_u_iou'][:, :H], g['bu_u_f'], g['bu_u_iou'][:, H:2*H])
    bbu = cat(g['bu_b_iou'][2*H:3*H], g['bu_b_iou'][:H], g['bu_b_f'], g['bu_b_iou'][H:2*H])
    def td_re(w):
        return cat(w[..., 2*H:3*H], w[..., :H], w[..., H:2*H], w[..., 3*H:])
    Wtd = td_re(g['td_w_ih']); Utd = td_re(g['td_w_hh'])
    btd = td_re(g['td_b_ih'] + g['td_b_hh'])
    WpreBU = g['enc_w2'] @ Wbu; bpreBU = g['enc_b2'] @ Wbu + bbu
    WpreTD = g['enc_w2'] @ Wtd; bpreTD = g['enc_b2'] @ Wtd + btd

    def ktile(w):  # [256, C] -> [128, 2, C]
        return np.ascontiguousarray(w.reshape(2, 128, -1).transpose(1, 0, 2))

    return {
        'w1': np.ascontiguousarray(g['enc_w1']).astype(bf16),
        'b1t': np.ascontiguousarray(g['enc_b1'].reshape(2, 128).T).astype(np.float32),
        'WpreBU': ktile(WpreBU).astype(bf16), 'WpreTD': ktile(WpreTD).astype(bf16),
        'UBU': ktile(Ubu).astype(bf16), 'UTD': ktile(Utd).astype(bf16),
        'bpreBU': bpreBU.reshape(1, -1).astype(bf16), 'bpreTD': bpreTD.reshape(1, -1).astype(bf16),
        'dw1': np.ascontiguousarray(g['dec_w1'].reshape(4, 128, H).transpose(1, 0, 2)).astype(bf16),
        'db1': g['dec_b1'].reshape(1, -1).astype(bf16),
        'dw2': ktile(g['dec_w2']).astype(bf16),
        'db2': g['dec_b2'].reshape(1, -1).astype(bf16),
    }


def _pack_x(x_core):
    parts = [x_core[:, 0, :, :].transpose(1, 0, 2).reshape(NP * b, IN),
             x_core[:, 1, :CS[0], :].transpose(1, 0, 2).reshape(CS[0] * b, IN),
             x_core[:, 2, :CS[1], :].transpose(1, 0, 2).reshape(CS[1] * b, IN)]
    xs = np.concatenate(parts, 0)
    return np.ascontiguousarray(xs.T).astype(bf16)


def build_nc():
    import concourse.bass as bass
    import concourse.mybir as mybir
    import concourse.tile as tile
    from concourse import bacc
    from contextlib import ExitStack

    dt = mybir.dt
    AF = mybir.ActivationFunctionType
    AP = bass.AP

    nc = bacc.Bacc(None, target_bir_lowering=False)
    P = {}
    spec = [('xT', (IN, NCOL), dt.bfloat16), ('w1', (IN, H), dt.bfloat16),
            ('b1t', (128, 2), dt.float32),
            ('WpreBU', (128, 2, 4*H), dt.bfloat16), ('WpreTD', (128, 2, 4*H), dt.bfloat16),
            ('UBU', (128, 2, 4*H), dt.bfloat16), ('UTD', (128, 2, 4*H), dt.bfloat16),
            ('bpreBU', (1, 4*H), dt.bfloat16), ('bpreTD', (1, 4*H), dt.bfloat16),
            ('dw1', (128, 4, H), dt.bfloat16), ('db1', (1, H), dt.bfloat16),
            ('dw2', (128, 2, 3), dt.bfloat16), ('db2', (1, 3), dt.bfloat16)]
    for n, shape, d in spec:
        P[n] = nc.declare_dram_parameter(n, list(shape), d, isOutput=False)
    out_d = nc.declare_dram_parameter('out', [3, NCOL], dt.float32, isOutput=True)

    def full(handle, rank):
        return handle[tuple(slice(None) for _ in range(rank))]

    with tile.TileContext(nc) as tc, ExitStack() as ctx:
        sing = ctx.enter_context(tc.tile_pool(name="sing", bufs=1))
        xin = ctx.enter_context(tc.tile_pool(name="xin", bufs=3))
        gpool = ctx.enter_context(tc.tile_pool(name="gpool", bufs=3))
        tpool = ctx.enter_context(tc.tile_pool(name="tpool", bufs=3))
        cpool = ctx.enter_context(tc.tile_pool(name="cpool", bufs=2))
        opool = ctx.enter_context(tc.tile_pool(name="opool", bufs=3))

        W = {}
        for n, shape, d in spec:
            if n == 'xT':
                continue
            t = sing.tile(list(shape), d, tag=f"w_{n}")
            nc.sync.dma_start(out=t, in_=full(P[n], len(shape)))
            W[n] = t
        ones = sing.tile([1, 512], dt.bfloat16, tag="ones")
        nc.vector.memset(ones, 1.0)
        zrow = sing.tile([1, 1024], dt.bfloat16, tag="zrow")
        nc.vector.memset(zrow, 0.0)

        h1 = sing.tile([128, 2, NCOL], dt.bfloat16, tag="h1")
        bu_h = sing.tile([128, 2, NCOL], dt.bfloat16, tag="bu_h")
        td_h = sing.tile([128, 2, NCOL], dt.bfloat16, tag="td_h")
        croots = sing.tile([128, 2, 128], dt.float32, tag="croots")
        tdseed = sing.tile([128, 2, 128], dt.float32, tag="tdseed")

        # ---------------- encoder ----------------
        with tc.tile_pool(name="eps", bufs=2, space="PSUM") as eps:
            for gi in range(NG):
                xg = xin.tile([IN, 512], dt.bfloat16, tag="xg")
                nc.sync.dma_start(out=xg, in_=P['xT'][:, gi*512:(gi+1)*512])
                pe = eps.tile([128, 2, 512], dt.float32, tag="pe")
                for t in range(2):
                    nc.tensor.matmul(pe[:, t, :], W['w1'][:, t*128:(t+1)*128], xg,
                                     start=True, stop=True)
                for t in range(2):
                    nc.scalar.activation(h1[:, t, gi*512:(gi+1)*512], pe[:, t, :],
                                         AF.Relu, bias=W['b1t'][:, t:t+1])

        # ---------------- helpers ----------------
        def hslice(buf, k, grp):
            """[128, 64] AP: buf[:, k, grp*b:(grp+1)*b]"""
            return buf[:, k, grp*b:(grp+1)*b]

        def hpair(buf, k, grp_lo, gstep):
            """[128, 2, 64] AP over groups {grp_lo, grp_lo+gstep}"""
            base = buf[:, k, :]
            return AP(tensor=base.tensor, offset=base.offset + grp_lo * b,
                      ap=[base.ap[0], [gstep * b, 2], [1, b]])

        def store_ap(buf, grp_lo, gstep, nbr):
            """[128, 2, nbr, 64] write AP into both hh halves of buf."""
            return AP(tensor=buf.tensor, offset=buf.offset + grp_lo * b,
                      ap=[buf.ap[0], [NCOL, 2], [gstep * b, nbr] if nbr > 1 else [0, 1],
                          [1, b]])

        def pre_window(ps, Wpre, bpre, rhs_fn, ncolw):
            cpb = 2048 // (ncolw * 4)       # chunks per PSUM bank
            for j in range(8):
                dst = ps[:, j, 0:ncolw]
                for k in range(2):
                    nc.tensor.matmul(dst, Wpre[:, k, j*128:(j+1)*128], rhs_fn(k),
                                     start=(k == 0 and j % cpb == 0), stop=False,
                                     skip_group_check=True)
                nc.tensor.matmul(dst, bpre[:, j*128:(j+1)*128], ones[:, 0:ncolw],
                                 start=False, stop=False, skip_group_check=True)

        def emit_step(ps, off, n, U, h_aps, c_prev, store, extra=None):
            """One LSTM step on psum cols [off, off+n). h_aps None => leaf step.
            c_prev: [128, 2, n] AP or None. store: [128, 2, nbr, 64] AP.
            extra: (eh_aps, ec_ap, fe_pool, pre_rhs) coupling extra child."""
            sl = lambda j: ps[:, j, off:off+n]
            if extra is not None:
                eh_aps = extra[0]
                for j in (0, 1, 2, 3, 6, 7):
                    for k in range(2):
                        nc.tensor.matmul(sl(j), U[:, k, j*128:(j+1)*128], eh_aps[k],
                                         start=False, stop=False, skip_group_check=True)
            if h_aps is not None:
                for j in range(8):
                    for k in range(2):
                        nc.tensor.matmul(sl(j), U[:, k, j*128:(j+1)*128], h_aps[k],
                                         start=False, stop=(k == 1), skip_group_check=True)
            else:
                for j in range(8):
                    nc.tensor.matmul(sl(j), zrow[:, j*128:(j+1)*128], ones[:, 0:n],
                                     start=False, stop=True, skip_group_check=True)
            gates = gpool.tile([128, 8, 128], dt.float32, tag="gates")
            g = lambda j0, j1: gates[:, j0:j1, 0:n]
            pin = lambda j0, j1: ps[:, j0:j1, off:off+n]
            nc.scalar.activation(g(0, 2), pin(0, 2), AF.Tanh)
            nc.scalar.activation(g(2, 8), pin(2, 8), AF.Sigmoid)
            t1 = tpool.tile([128, 2, 128], dt.float32, tag="t1")
            nc.gpsimd.tensor_mul(t1[:, :, 0:n], g(2, 4), g(0, 2))
            c_new = cpool.tile([128, 2, 128], dt.float32, tag="c")
            cn = c_new[:, :, 0:n]
            if h_aps is None and c_prev is None:
                nc.vector.tensor_copy(cn, t1[:, :, 0:n])
            else:
                nc.vector.tensor_mul(cn, g(4, 6), c_prev)
                nc.vector.tensor_add(cn, cn, t1[:, :, 0:n])
            if extra is not None:
                nc.vector.tensor_add(cn, cn, extra[1])
            tct = tpool.tile([128, 2, 128], dt.float32, tag="tc")
            nc.scalar.activation(tct[:, :, 0:n], cn, AF.Tanh)
            nbr = n // b
            nc.vector.tensor_mul(
                store,
                g(6, 8).rearrange("p c (r s) -> p c r s", s=b),
                tct[:, :, 0:n].rearrange("p c (r s) -> p c r s", s=b))
            return cn

        AluOp = mybir.AluOpType

        # ------------- phase A: bu children, v = 47..0, slots ascend v -------------
        with tc.tile_pool(name="pgA", bufs=2, space="PSUM") as pgA:
            c_prev = None
            ps = None
            for s in range(CS[1]):
                v = CS[1] - 1 - s
                if s % WIN == 0:
                    ps = pgA.tile([128, 8, WIN * 128], dt.float32, tag="pg")
                    v_lo = v - (WIN - 1)
                    def rhs(k, v_lo=v_lo):
                        bb = h1[:, k, :]
                        return AP(tensor=bb.tensor, offset=bb.offset + (64 + v_lo) * b,
                                  ap=[bb.ap[0], [b, WIN], [32 * b, 2], [1, b]])
                    pre_window(ps, W['WpreBU'], W['bpreBU'], rhs, WIN * 128)
                slot_i = (WIN - 1) - (s % WIN)          # ascending v in window
                off = slot_i * 128
                if v > CS[0] - 1:
                    # c2 only; c2 occupies cols [64:128) of the slot
                    h_aps = None if s == 0 else tuple(hslice(bu_h, k, 96 + v + 1) for k in range(2))
                    store = store_ap(bu_h, 96 + v, 0, 1)
                    c_prev = emit_step(ps, off + 64, 64, W['UBU'], h_aps, c_prev, store)
                elif v == CS[0] - 1:
                    # c1 leaf joins: c1 half (cols 0:64) first-step, c2 half normal
                    h_aps = tuple(hslice(bu_h, k, 96 + v + 1) for k in range(2))
                    sl = lambda j: ps[:, j, off+64:off+128]
                    for j in range(8):
                        for k in range(2):
                            nc.tensor.matmul(sl(j), W['UBU'][:, k, j*128:(j+1)*128],
                                             h_aps[k], start=False, stop=(k == 1),
                                             skip_group_check=True)
                    gates = gpool.tile([128, 8, 128], dt.float32, tag="gates")
                    pin = lambda j0, j1: ps[:, j0:j1, off:off+128]
                    nc.scalar.activation(gates[:, 0:2, :], pin(0, 2), AF.Tanh)
                    nc.scalar.activation(gates[:, 2:8, :], pin(2, 8), AF.Sigmoid)
                    t1 = tpool.tile([128, 2, 128], dt.float32, tag="t1")
                    nc.gpsimd.tensor_mul(t1, gates[:, 2:4, :], gates[:, 0:2, :])
                    c_new = cpool.tile([128, 2, 128], dt.float32, tag="c")
                    nc.vector.tensor_copy(c_new[:, :, 0:64], t1[:, :, 0:64])
                    nc.vector.tensor_mul(c_new[:, :, 64:128], gates[:, 4:6, 64:128], c_prev)
                    nc.vector.tensor_add(c_new[:, :, 64:128], c_new[:, :, 64:128],
                                         t1[:, :, 64:128])
                    tct = tpool.tile([128, 2, 128], dt.float32, tag="tc")
                    nc.scalar.activation(tct, c_new, AF.Tanh)
                    store = store_ap(bu_h, 64 + v, 32, 2)
                    nc.vector.tensor_mul(
                        store,
                        gates[:, 6:8, :].rearrange("p c (r s) -> p c r s", s=b),
                        tct.rearrange("p c (r s) -> p c r s", s=b))
                    c_prev = c_new
                else:
                    h_aps = tuple(hpair(bu_h, k, 64 + v + 1, 32) for k in range(2))
                    store = store_ap(bu_h, 64 + v, 32, 2)
                    c_prev = emit_step(ps, off, 128, W['UBU'], h_aps, c_prev, store)
            nc.vector.tensor_copy(croots, c_prev)

        # precompute coupling extra-child c contribution: sig(pre_f + eh@Uf) * ec
        cextra = sing.tile([128, 2, 128], dt.float32, tag="cextra")
        with tc.tile_pool(name="feP", bufs=2, space="PSUM") as feP:
            for idx, (vv, grp) in enumerate([(COUP[0], 64), (COUP[1], 96)]):
                fe = feP.tile([128, 2, b], dt.float32, tag="fe")
                eh = tuple(hslice(bu_h, k, grp) for k in range(2))
                prr = tuple(hslice(h1, k, vv) for k in range(2))
                for hh in range(2):
                    jj = 4 + hh
                    for k in range(2):
                        nc.tensor.matmul(fe[:, hh, :], W['WpreBU'][:, k, jj*128:(jj+1)*128],
                                         prr[k], start=(hh == 0 and k == 0), stop=False,
                                         skip_group_check=True)
                    nc.tensor.matmul(fe[:, hh, :], W['bpreBU'][:, jj*128:(jj+1)*128],
                                     ones[:, 0:b], start=False, stop=False,
                                     skip_group_check=True)
                    for k in range(2):
                        nc.tensor.matmul(fe[:, hh, :], W['UBU'][:, k, jj*128:(jj+1)*128],
                                         eh[k], start=False,
                                         stop=(hh == 1 and k == 1), skip_group_check=True)
                fes = tpool.tile([128, 2, b], dt.float32, tag="fes")
                nc.scalar.activation(fes, fe, AF.Sigmoid)
                nc.vector.tensor_mul(cextra[:, :, idx*64:(idx+1)*64], fes,
                                     croots[:, :, idx*64:(idx+1)*64])

        # ------------- phase B: bu parent, v = 63..0 -------------
        WP = 4
        with tc.tile_pool(name="pgB", bufs=2, space="PSUM") as pgB:
            c_prev = None
            ps = None
            for s in range(NP):
                v = NP - 1 - s
                if s % WP == 0:
                    ps = pgB.tile([128, 8, WP * 64], dt.float32, tag="pg")
                    v_lo = v - (WP - 1)
                    def rhs(k, v_lo=v_lo):
                        bb = h1[:, k, :]
                        return AP(tensor=bb.tensor, offset=bb.offset + v_lo * b,
                                  ap=[bb.ap[0], [b, WP], [1, b]])
                    pre_window(ps, W['WpreBU'], W['bpreBU'], rhs, WP * 64)
                off = ((WP - 1) - (s % WP)) * 64
                extra = None
                if v == COUP[0]:
                    eh = tuple(hslice(bu_h, k, 64) for k in range(2))
                    extra = (eh, cextra[:, :, 0:64])
                elif v == COUP[1]:
                    eh = tuple(hslice(bu_h, k, 96) for k in range(2))
                    extra = (eh, cextra[:, :, 64:128])
                h_aps = None if s == 0 else tuple(hslice(bu_h, k, v + 1) for k in range(2))
                store = store_ap(bu_h, v, 0, 1)
                c_prev = emit_step(ps, off, 64, W['UBU'], h_aps, c_prev, store, extra=extra)

        nc.vector.tensor_copy(td_h[:, :, 0:b], bu_h[:, :, 0:b])

        # ------------- phase C: td parent, v = 1..63 -------------
        with tc.tile_pool(name="pgC", bufs=2, space="PSUM") as pgC:
            WP = 4
            ps = None
            for i, v in enumerate(range(1, NP)):
                if i % WP == 0:
                    ps = pgC.tile([128, 8, WP * 64], dt.float32, tag="pg")
                    def rhs(k, v0=v):
                        bb = h1[:, k, :]
                        return AP(tensor=bb.tensor, offset=bb.offset + v0 * b,
                                  ap=[bb.ap[0], [b, WP], [1, b]])
                    pre_window(ps, W['WpreTD'], W['bpreTD'], rhs, WP * 64)
                off = (i % WP) * 64
                h_aps = tuple(hslice(td_h, k, v - 1) for k in range(2))
                store = store_ap(td_h, v, 0, 1)
                c_prev = emit_step(ps, off, 64, W['UTD'], h_aps, c_prev, store)
                if v == COUP[0]:
                    nc.vector.tensor_copy(tdseed[:, :, 0:64], c_prev)
                elif v == COUP[1]:
                    nc.vector.tensor_copy(tdseed[:, :, 64:128], c_prev)

        # ------------- phase D: td children, v = 0..47 -------------
        with tc.tile_pool(name="pgD", bufs=2, space="PSUM") as pgD:
            c_prev = None
            ps = None
            for s in range(CS[1]):
                v = s
                if s % WIN == 0:
                    ps = pgD.tile([128, 8, WIN * 128], dt.float32, tag="pg")
                    def rhs(k, v0=v):
                        bb = h1[:, k, :]
                        return AP(tensor=bb.tensor, offset=bb.offset + (64 + v0) * b,
                                  ap=[bb.ap[0], [b, WIN], [32 * b, 2], [1, b]])
                    pre_window(ps, W['WpreTD'], W['bpreTD'], rhs, WIN * 128)
                off = (s % WIN) * 128
                if s == 0:
                    h_aps = tuple(
                        AP(tensor=td_h[:, k, :].tensor,
                           offset=td_h[:, k, :].offset + COUP[0] * b,
                           ap=[td_h[:, k, :].ap[0], [(COUP[1] - COUP[0]) * b, 2], [1, b]])
                        for k in range(2))
                    store = store_ap(td_h, 64 + v, 32, 2)
                    c_prev = emit_step(ps, off, 128, W['UTD'], h_aps, tdseed, store)
                elif v < CS[0]:
                    h_aps = tuple(hpair(td_h, k, 64 + v - 1, 32) for k in range(2))
                    store = store_ap(td_h, 64 + v, 32, 2)
                    c_prev = emit_step(ps, off, 128, W['UTD'], h_aps, c_prev, store)
                else:
                    h_aps = tuple(hslice(td_h, k, 96 + v - 1) for k in range(2))
                    store = store_ap(td_h, 96 + v, 0, 1)
                    cp = c_prev[:, :, 64:128] if v == CS[0] else c_prev
                    c_prev = emit_step(ps, off + 64, 64, W['UTD'], h_aps, cp, store)

        # ------------- decoder -------------
        with tc.tile_pool(name="dps", bufs=2, space="PSUM") as dps, \
             tc.tile_pool(name="ops", bufs=2, space="PSUM") as ops:
            for gi in range(NG):
                cs0, cs1 = gi * 512, (gi + 1) * 512
                pd = dps.tile([128, 2, 512], dt.float32, tag="pd")
                for m in range(2):
                    for k in range(4):
                        srcbuf = bu_h if k < 2 else td_h
                        nc.tensor.matmul(pd[:, m, :], W['dw1'][:, k, m*128:(m+1)*128],
                                         srcbuf[:, k % 2, cs0:cs1],
                                         start=(k == 0), stop=False)
                    nc.tensor.matmul(pd[:, m, :], W['db1'][:, m*128:(m+1)*128],
                                     ones[:, 0:512], start=False, stop=True)
                h2 = opool.tile([128, 2, 512], dt.bfloat16, tag="h2")
                for m in range(2):
                    nc.scalar.activation(h2[:, m, :], pd[:, m, :], AF.Relu)
                po = ops.tile([3, 512], dt.float32, tag="po")
                for k in range(2):
                    nc.tensor.matmul(po, W['dw2'][:, k, :], h2[:, k, :],
                                     start=(k == 0), stop=False)
                nc.tensor.matmul(po, W['db2'], ones[:, 0:512], start=False, stop=True)
                og = opool.tile([3, 512], dt.float32, tag="og")
                nc.vector.tensor_copy(og, po)
                nc.sync.dma_start(out=out_d[:, cs0:cs1], in_=og)

    nc.finalize()
    return nc


def kernel(**inputs):
    from concourse.bass_utils import run_bass_kernel_spmd

    g = {k: np.asarray(v, np.float32) for k, v in inputs.items()}
    x = g['x']
    Wd = _fuse_weights(g)

    if 'nc' not in _cache:
        _cache['nc'] = build_nc()
    nc = _cache['nc']

    in_maps = []
    for c in range(NCORES):
        m = dict(Wd)
        m['xT'] = _pack_x(x[c*b:(c+1)*b])
        in_maps.append(m)
    res = run_bass_kernel_spmd(nc, in_maps, list(range(NCORES)))

    const = (np.maximum(g['dec_b1'], 0) @ g['dec_w2'] + g['dec_b2']).astype(np.float32)
    out = np.empty((B, NB, NP, 3), np.float32)
    out[:] = const
    for c in range(NCORES):
        o = np.asarray(res.results[c]['out'])      # [3, 9216]
        sl = o.T.reshape(NSLOT, b, 3).transpose(1, 0, 2)  # [sample, slot, 3]
        out[c*b:(c+1)*b, 0, :, :] = sl[:, :NP]
        out[c*b:(c+1)*b, 1, :CS[0], :] = sl[:, NP:NP+CS[0]]
        out[c*b:(c+1)*b, 2, :CS[1], :] = sl[:, NP+CS[0]:]
    return out


# revision 7
# speedup vs baseline: 1.1121x; 1.0054x over previous
"""Trainium2 Bass kernel for nn_BDLOTreeLSTM_78847009620604.

8-core data parallelism over batch (64 samples/core). Layout: [dim-partitions,
node-cols]; H=256 -> 2 partition halves; 144 (branch, vertex) slots x 64
samples = 9216 node columns per core (parent v -> slot v, c1 v -> 64+v,
c2 v -> 96+v).

- encoder L2 is folded into the pre-gate weights on host:
  pre = h1 @ (enc_w2 @ Wgate) + (enc_b2 @ Wgate + bgate),  h1 = relu(x@w1+b1)
- chain steps: pre-gates are accumulated into PSUM by windowed matmuls (bf16),
  the recurrent h @ U term accumulates on top; gate chunk order [i, u, f, o];
  sigmoid/tanh on ScalarE reading PSUM; cell update on VectorE (+ i*u on
  GPSIMD); h written in bf16 directly into the big state buffers.
- child chains c1/c2 run batched as one stream (slot cols [c1 | c2]).
- biases enter via rank-1 ones-row matmuls (K=1).
"""
import numpy as np
import ml_dtypes

B, H, NP, IN = 512, 256, 64, 9
CS = (32, 48)
COUP = (16, 40)
NB = 3
NCORES = 8
b = B // NCORES                  # 64
NSLOT = NP + CS[0] + CS[1]       # 144
NCOL = NSLOT * b                 # 9216
NG = NCOL // 512                 # 18
WIN = 2                          # chain pre-gate window (steps)

bf16 = ml_dtypes.bfloat16
_cache = {}


def _fuse_weights(g):
    def cat(*xs):
        return np.concatenate(xs, -1)
    Wbu = cat(g['bu_w_iou'][:, 2*H:3*H], g['bu_w_iou'][:, :H], g['bu_w_f'], g['bu_w_iou'][:, H:2*H])
    Ubu = cat(g['bu_u_iou'][:, 2*H:3*H], g['bu_u_iou'][:, :H], g['bu_u_f'], g['bu_u_iou'][:, H:2*H])
    bbu = cat(g['bu_b_iou'][2*H:3*H], g['bu_b_iou'][:H], g['bu_b_f'], g['bu_b_iou'][H:2*H])
    def td_re(w):
        return cat(w[..., 2*H:3*H], w[..., :H], w[..., H:2*H], w[..., 3*H:])
    Wtd = td_re(g['td_w_ih']); Utd = td_re(g['td_w_hh'])
    btd = td_re(g['td_b_ih'] + g['td_b_hh'])
    WpreBU = g['enc_w2'] @ Wbu; bpreBU = g['enc_b2'] @ Wbu + bbu
    WpreTD = g['enc_w2'] @ Wtd; bpreTD = g['enc_b2'] @ Wtd + btd

    def ktile(w):  # [256, C] -> [128, 2, C]
        return np.ascontiguousarray(w.reshape(2, 128, -1).transpose(1, 0, 2))

    return {
        'w1': np.ascontiguousarray(g['enc_w1']).astype(bf16),
        'b1t': np.ascontiguousarray(g['enc_b1'].reshape(2, 128).T).astype(np.float32),
        'WpreBU': ktile(WpreBU).astype(bf16), 'WpreTD': ktile(WpreTD).astype(bf16),
        'UBU': ktile(Ubu).astype(bf16), 'UTD': ktile(Utd).astype(bf16),
        'bpreBU': bpreBU.reshape(1, -1).astype(bf16), 'bpreTD': bpreTD.reshape(1, -1).astype(bf16),
        'dw1': np.ascontiguousarray(g['dec_w1'].reshape(4, 128, H).transpose(1, 0, 2)).astype(bf16),
        'db1': g['dec_b1'].reshape(1, -1).astype(bf16),
        'dw2': ktile(g['dec_w2']).astype(bf16),
        'db2': g['dec_b2'].reshape(1, -1).astype(bf16),
    }


def _pack_x(x_core):
    parts = [x_core[:, 0, :, :].transpose(1, 0, 2).reshape(NP * b, IN),
             x_core[:, 1, :CS[0], :].transpose(1, 0, 2).reshape(CS[0] * b, IN),
             x_core[:, 2, :CS[1], :].transpose(1, 0, 2).reshape(CS[1] * b, IN)]
    xs = np.concatenate(parts, 0)
    return np.ascontiguousarray(xs.T).astype(bf16)


def build_nc():
    import concourse.bass as bass
    import concourse.mybir as mybir
    import concourse.tile as tile
    from concourse import bacc
    from contextlib import ExitStack

    dt = mybir.dt
    AF = mybir.ActivationFunctionType
    AP = bass.AP

    nc = bacc.Bacc(None, target_bir_lowering=False)
    P = {}
    spec = [('xT', (IN, NCOL), dt.bfloat16), ('w1', (IN, H), dt.bfloat16),
            ('b1t', (128, 2), dt.float32),
            ('WpreBU', (128, 2, 4*H), dt.bfloat16), ('WpreTD', (128, 2, 4*H), dt.bfloat16),
            ('UBU', (128, 2, 4*H), dt.bfloat16), ('UTD', (128, 2, 4*H), dt.bfloat16),
            ('bpreBU', (1, 4*H), dt.bfloat16), ('bpreTD', (1, 4*H), dt.bfloat16),
            ('dw1', (128, 4, H), dt.bfloat16), ('db1', (1, H), dt.bfloat16),
            ('dw2', (128, 2, 3), dt.bfloat16), ('db2', (1, 3), dt.bfloat16)]
    for n, shape, d in spec:
        P[n] = nc.declare_dram_parameter(n, list(shape), d, isOutput=False)
    out_d = nc.declare_dram_parameter('out', [3, NCOL], dt.float32, isOutput=True)

    def full(handle, rank):
        return handle[tuple(slice(None) for _ in range(rank))]

    with tile.TileContext(nc) as tc, ExitStack() as ctx:
        sing = ctx.enter_context(tc.tile_pool(name="sing", bufs=1))
        xin = ctx.enter_context(tc.tile_pool(name="xin", bufs=3))
        gpool = ctx.enter_context(tc.tile_pool(name="gpool", bufs=3))
        tpool = ctx.enter_context(tc.tile_pool(name="tpool", bufs=3))
        cpool = ctx.enter_context(tc.tile_pool(name="cpool", bufs=2))
        opool = ctx.enter_context(tc.tile_pool(name="opool", bufs=3))

        W = {}
        for n, shape, d in spec:
            if n == 'xT':
                continue
            t = sing.tile(list(shape), d, tag=f"w_{n}")
            nc.sync.dma_start(out=t, in_=full(P[n], len(shape)))
            W[n] = t
        ones = sing.tile([1, 512], dt.bfloat16, tag="ones")
        nc.vector.memset(ones, 1.0)
        zrow = sing.tile([1, 1024], dt.bfloat16, tag="zrow")
        nc.vector.memset(zrow, 0.0)

        h1 = sing.tile([128, 2, NCOL], dt.bfloat16, tag="h1")
        bu_h = sing.tile([128, 2, NCOL], dt.bfloat16, tag="bu_h")
        td_h = sing.tile([128, 2, NCOL], dt.bfloat16, tag="td_h")
        croots = sing.tile([128, 2, 128], dt.float32, tag="croots")
        tdseed = sing.tile([128, 2, 128], dt.float32, tag="tdseed")

        # ---------------- encoder ----------------
        with tc.tile_pool(name="eps", bufs=2, space="PSUM") as eps:
            for gi in range(NG - 1, -1, -1):   # children cols first (phase A needs them)
                xg = xin.tile([IN, 512], dt.bfloat16, tag="xg")
                nc.sync.dma_start(out=xg, in_=P['xT'][:, gi*512:(gi+1)*512])
                pe = eps.tile([128, 2, 512], dt.float32, tag="pe")
                for t in range(2):
                    nc.tensor.matmul(pe[:, t, :], W['w1'][:, t*128:(t+1)*128], xg,
                                     start=True, stop=True)
                for t in range(2):
                    nc.scalar.activation(h1[:, t, gi*512:(gi+1)*512], pe[:, t, :],
                                         AF.Relu, bias=W['b1t'][:, t:t+1])

        # ---------------- helpers ----------------
        def hslice(buf, k, grp):
            """[128, 64] AP: buf[:, k, grp*b:(grp+1)*b]"""
            return buf[:, k, grp*b:(grp+1)*b]

        def hpair(buf, k, grp_lo, gstep):
            """[128, 2, 64] AP over groups {grp_lo, grp_lo+gstep}"""
            base = buf[:, k, :]
            return AP(tensor=base.tensor, offset=base.offset + grp_lo * b,
                      ap=[base.ap[0], [gstep * b, 2], [1, b]])

        def store_ap(buf, grp_lo, gstep, nbr):
            """[128, 2, nbr, 64] write AP into both hh halves of buf."""
            return AP(tensor=buf.tensor, offset=buf.offset + grp_lo * b,
                      ap=[buf.ap[0], [NCOL, 2], [gstep * b, nbr] if nbr > 1 else [0, 1],
                          [1, b]])

        def pre_window(ps, Wpre, bpre, rhs_fn, ncolw):
            cpb = 2048 // (ncolw * 4)       # chunks per PSUM bank
            for j in range(8):
                dst = ps[:, j, 0:ncolw]
                for k in range(2):
                    nc.tensor.matmul(dst, Wpre[:, k, j*128:(j+1)*128], rhs_fn(k),
                                     start=(k == 0 and j % cpb == 0), stop=False,
                                     skip_group_check=True)
                nc.tensor.matmul(dst, bpre[:, j*128:(j+1)*128], ones[:, 0:ncolw],
                                 start=False, stop=False, skip_group_check=True)

        def emit_step(ps, off, n, U, h_aps, c_prev, store, extra=None):
            """One LSTM step on psum cols [off, off+n). h_aps None => leaf step.
            c_prev: [128, 2, n] AP or None. store: [128, 2, nbr, 64] AP.
            extra: (eh_aps, ec_ap, fe_pool, pre_rhs) coupling extra child."""
            sl = lambda j: ps[:, j, off:off+n]
            if extra is not None:
                eh_aps = extra[0]
                for j in (0, 1, 2, 3, 6, 7):
                    for k in range(2):
                        nc.tensor.matmul(sl(j), U[:, k, j*128:(j+1)*128], eh_aps[k],
                                         start=False, stop=False, skip_group_check=True)
            if h_aps is not None:
                for j in range(8):
                    for k in range(2):
                        nc.tensor.matmul(sl(j), U[:, k, j*128:(j+1)*128], h_aps[k],
                                         start=False, stop=(k == 1), skip_group_check=True)
            else:
                for j in range(8):
                    nc.tensor.matmul(sl(j), zrow[:, j*128:(j+1)*128], ones[:, 0:n],
                                     start=False, stop=True, skip_group_check=True)
            gates = gpool.tile([128, 8, 128], dt.float32, tag="gates")
            g = lambda j0, j1: gates[:, j0:j1, 0:n]
            pin = lambda j0, j1: ps[:, j0:j1, off:off+n]
            nc.scalar.activation(g(0, 2), pin(0, 2), AF.Tanh)
            nc.scalar.activation(g(2, 8), pin(2, 8), AF.Sigmoid)
            t1 = tpool.tile([128, 2, 128], dt.float32, tag="t1")
            nc.gpsimd.tensor_mul(t1[:, :, 0:n], g(2, 4), g(0, 2))
            c_new = cpool.tile([128, 2, 128], dt.float32, tag="c")
            cn = c_new[:, :, 0:n]
            if h_aps is None and c_prev is None:
                nc.vector.tensor_copy(cn, t1[:, :, 0:n])
            else:
                nc.vector.tensor_mul(cn, g(4, 6), c_prev)
                nc.vector.tensor_add(cn, cn, t1[:, :, 0:n])
            if extra is not None:
                nc.vector.tensor_add(cn, cn, extra[1])
            tct = tpool.tile([128, 2, 128], dt.float32, tag="tc")
            nc.scalar.activation(tct[:, :, 0:n], cn, AF.Tanh)
            nbr = n // b
            nc.vector.tensor_mul(
                store,
                g(6, 8).rearrange("p c (r s) -> p c r s", s=b),
                tct[:, :, 0:n].rearrange("p c (r s) -> p c r s", s=b))
            return cn

        AluOp = mybir.AluOpType

        # ------------- phase A: bu children, v = 47..0, slots ascend v -------------
        with tc.tile_pool(name="pgA", bufs=2, space="PSUM") as pgA:
            c_prev = None
            ps = None
            for s in range(CS[1]):
                v = CS[1] - 1 - s
                if s % WIN == 0:
                    ps = pgA.tile([128, 8, WIN * 128], dt.float32, tag="pg")
                    v_lo = v - (WIN - 1)
                    def rhs(k, v_lo=v_lo):
                        bb = h1[:, k, :]
                        return AP(tensor=bb.tensor, offset=bb.offset + (64 + v_lo) * b,
                                  ap=[bb.ap[0], [b, WIN], [32 * b, 2], [1, b]])
                    pre_window(ps, W['WpreBU'], W['bpreBU'], rhs, WIN * 128)
                slot_i = (WIN - 1) - (s % WIN)          # ascending v in window
                off = slot_i * 128
                if v > CS[0] - 1:
                    # c2 only; c2 occupies cols [64:128) of the slot
                    h_aps = None if s == 0 else tuple(hslice(bu_h, k, 96 + v + 1) for k in range(2))
                    store = store_ap(bu_h, 96 + v, 0, 1)
                    c_prev = emit_step(ps, off + 64, 64, W['UBU'], h_aps, c_prev, store)
                elif v == CS[0] - 1:
                    # c1 leaf joins: c1 half (cols 0:64) first-step, c2 half normal
                    h_aps = tuple(hslice(bu_h, k, 96 + v + 1) for k in range(2))
                    sl = lambda j: ps[:, j, off+64:off+128]
                    for j in range(8):
                        for k in range(2):
                            nc.tensor.matmul(sl(j), W['UBU'][:, k, j*128:(j+1)*128],
                                             h_aps[k], start=False, stop=(k == 1),
                                             skip_group_check=True)
                    gates = gpool.tile([128, 8, 128], dt.float32, tag="gates")
                    pin = lambda j0, j1: ps[:, j0:j1, off:off+128]
                    nc.scalar.activation(gates[:, 0:2, :], pin(0, 2), AF.Tanh)
                    nc.scalar.activation(gates[:, 2:8, :], pin(2, 8), AF.Sigmoid)
                    t1 = tpool.tile([128, 2, 128], dt.float32, tag="t1")
                    nc.gpsimd.tensor_mul(t1, gates[:, 2:4, :], gates[:, 0:2, :])
                    c_new = cpool.tile([128, 2, 128], dt.float32, tag="c")
                    nc.vector.tensor_copy(c_new[:, :, 0:64], t1[:, :, 0:64])
                    nc.vector.tensor_mul(c_new[:, :, 64:128], gates[:, 4:6, 64:128], c_prev)
                    nc.vector.tensor_add(c_new[:, :, 64:128], c_new[:, :, 64:128],
                                         t1[:, :, 64:128])
                    tct = tpool.tile([128, 2, 128], dt.float32, tag="tc")
                    nc.scalar.activation(tct, c_new, AF.Tanh)
                    store = store_ap(bu_h, 64 + v, 32, 2)
                    nc.vector.tensor_mul(
                        store,
                        gates[:, 6:8, :].rearrange("p c (r s) -> p c r s", s=b),
                        tct.rearrange("p c (r s) -> p c r s", s=b))
                    c_prev = c_new
                else:
                    h_aps = tuple(hpair(bu_h, k, 64 + v + 1, 32) for k in range(2))
                    store = store_ap(bu_h, 64 + v, 32, 2)
                    c_prev = emit_step(ps, off, 128, W['UBU'], h_aps, c_prev, store)
            nc.vector.tensor_copy(croots, c_prev)

        # precompute coupling extra-child c contribution: sig(pre_f + eh@Uf) * ec
        cextra = sing.tile([128, 2, 128], dt.float32, tag="cextra")
        with tc.tile_pool(name="feP", bufs=2, space="PSUM") as feP:
            for idx, (vv, grp) in enumerate([(COUP[0], 64), (COUP[1], 96)]):
                fe = feP.tile([128, 2, b], dt.float32, tag="fe")
                eh = tuple(hslice(bu_h, k, grp) for k in range(2))
                prr = tuple(hslice(h1, k, vv) for k in range(2))
                for hh in range(2):
                    jj = 4 + hh
                    for k in range(2):
                        nc.tensor.matmul(fe[:, hh, :], W['WpreBU'][:, k, jj*128:(jj+1)*128],
                                         prr[k], start=(hh == 0 and k == 0), stop=False,
                                         skip_group_check=True)
                    nc.tensor.matmul(fe[:, hh, :], W['bpreBU'][:, jj*128:(jj+1)*128],
                                     ones[:, 0:b], start=False, stop=False,
                                     skip_group_check=True)
                    for k in range(2):
                        nc.tensor.matmul(fe[:, hh, :], W['UBU'][:, k, jj*128:(jj+1)*128],
                                         eh[k], start=False,
                                         stop=(hh == 1 and k == 1), skip_group_check=True)
                fes = tpool.tile([128, 2, b], dt.float32, tag="fes")
                nc.scalar.activation(fes, fe, AF.Sigmoid)
                nc.vector.tensor_mul(cextra[:, :, idx*64:(idx+1)*64], fes,
                                     croots[:, :, idx*64:(idx+1)*64])

        # ------------- phase B: bu parent, v = 63..0 -------------
        WP = 4
        with tc.tile_pool(name="pgB", bufs=2, space="PSUM") as pgB:
            c_prev = None
            ps = None
            for s in range(NP):
                v = NP - 1 - s
                if s % WP == 0:
                    ps = pgB.tile([128, 8, WP * 64], dt.float32, tag="pg")
                    v_lo = v - (WP - 1)
                    def rhs(k, v_lo=v_lo):
                        bb = h1[:, k, :]
                        return AP(tensor=bb.tensor, offset=bb.offset + v_lo * b,
                                  ap=[bb.ap[0], [b, WP], [1, b]])
                    pre_window(ps, W['WpreBU'], W['bpreBU'], rhs, WP * 64)
                off = ((WP - 1) - (s % WP)) * 64
                extra = None
                if v == COUP[0]:
                    eh = tuple(hslice(bu_h, k, 64) for k in range(2))
                    extra = (eh, cextra[:, :, 0:64])
                elif v == COUP[1]:
                    eh = tuple(hslice(bu_h, k, 96) for k in range(2))
                    extra = (eh, cextra[:, :, 64:128])
                h_aps = None if s == 0 else tuple(hslice(bu_h, k, v + 1) for k in range(2))
                store = store_ap(bu_h, v, 0, 1)
                c_prev = emit_step(ps, off, 64, W['UBU'], h_aps, c_prev, store, extra=extra)

        nc.vector.tensor_copy(td_h[:, :, 0:b], bu_h[:, :, 0:b])

        # ------------- phase C: td parent, v = 1..63 -------------
        with tc.tile_pool(name="pgC", bufs=2, space="PSUM") as pgC:
            WP = 4
            ps = None
            for i, v in enumerate(range(1, NP)):
                if i % WP == 0:
                    ps = pgC.tile([128, 8, WP * 64], dt.float32, tag="pg")
                    def rhs(k, v0=v):
                        bb = h1[:, k, :]
                        return AP(tensor=bb.tensor, offset=bb.offset + v0 * b,
                                  ap=[bb.ap[0], [b, WP], [1, b]])
                    pre_window(ps, W['WpreTD'], W['bpreTD'], rhs, WP * 64)
                off = (i % WP) * 64
                h_aps = tuple(hslice(td_h, k, v - 1) for k in range(2))
                store = store_ap(td_h, v, 0, 1)
                c_prev = emit_step(ps, off, 64, W['UTD'], h_aps, c_prev, store)
                if v == COUP[0]:
                    nc.vector.tensor_copy(tdseed[:, :, 0:64], c_prev)
                elif v == COUP[1]:
                    nc.vector.tensor_copy(tdseed[:, :, 64:128], c_prev)

        # ------------- phase D: td children, v = 0..47 -------------
        with tc.tile_pool(name="pgD", bufs=2, space="PSUM") as pgD:
            c_prev = None
            ps = None
            for s in range(CS[1]):
                v = s
                if s % WIN == 0:
                    ps = pgD.tile([128, 8, WIN * 128], dt.float32, tag="pg")
                    def rhs(k, v0=v):
                        bb = h1[:, k, :]
                        return AP(tensor=bb.tensor, offset=bb.offset + (64 + v0) * b,
                                  ap=[bb.ap[0], [b, WIN], [32 * b, 2], [1, b]])
                    pre_window(ps, W['WpreTD'], W['bpreTD'], rhs, WIN * 128)
                off = (s % WIN) * 128
                if s == 0:
                    h_aps = tuple(
                        AP(tensor=td_h[:, k, :].tensor,
                           offset=td_h[:, k, :].offset + COUP[0] * b,
                           ap=[td_h[:, k, :].ap[0], [(COUP[1] - COUP[0]) * b, 2], [1, b]])
                        for k in range(2))
                    store = store_ap(td_h, 64 + v, 32, 2)
                    c_prev = emit_step(ps, off, 128, W['UTD'], h_aps, tdseed, store)
                elif v < CS[0]:
                    h_aps = tuple(hpair(td_h, k, 64 + v - 1, 32) for k in range(2))
                    store = store_ap(td_h, 64 + v, 32, 2)
                    c_prev = emit_step(ps, off, 128, W['UTD'], h_aps, c_prev, store)
                else:
                    h_aps = tuple(hslice(td_h, k, 96 + v - 1) for k in range(2))
                    store = store_ap(td_h, 96 + v, 0, 1)
                    cp = c_prev[:, :, 64:128] if v == CS[0] else c_prev
                    c_prev = emit_step(ps, off + 64, 64, W['UTD'], h_aps, cp, store)

        # ------------- decoder -------------
        with tc.tile_pool(name="dps", bufs=2, space="PSUM") as dps, \
             tc.tile_pool(name="ops", bufs=2, space="PSUM") as ops:
            for gi in range(NG):
                cs0, cs1 = gi * 512, (gi + 1) * 512
                pd = dps.tile([128, 2, 512], dt.float32, tag="pd")
                for m in range(2):
                    for k in range(4):
                        srcbuf = bu_h if k < 2 else td_h
                        nc.tensor.matmul(pd[:, m, :], W['dw1'][:, k, m*128:(m+1)*128],
                                         srcbuf[:, k % 2, cs0:cs1],
                                         start=(k == 0), stop=False)
                    nc.tensor.matmul(pd[:, m, :], W['db1'][:, m*128:(m+1)*128],
                                     ones[:, 0:512], start=False, stop=True)
                h2 = opool.tile([128, 2, 512], dt.bfloat16, tag="h2")
                for m in range(2):
                    nc.scalar.activation(h2[:, m, :], pd[:, m, :], AF.Relu)
                po = ops.tile([3, 512], dt.float32, tag="po")
                for k in range(2):
                    nc.tensor.matmul(po, W['dw2'][:, k, :], h2[:, k, :],
                                     start=(k == 0), stop=False)
                nc.tensor.matmul(po, W['db2'], ones[:, 0:512], start=False, stop=True)
                og = opool.tile([3, 512], dt.float32, tag="og")
                nc.vector.tensor_copy(og, po)
                nc.sync.dma_start(out=out_d[:, cs0:cs1], in_=og)

    nc.finalize()
    return nc


def kernel(**inputs):
    from concourse.bass_utils import run_bass_kernel_spmd

    g = {k: np.asarray(v, np.float32) for k, v in inputs.items()}
    x = g['x']
    Wd = _fuse_weights(g)

    if 'nc' not in _cache:
        _cache['nc'] = build_nc()
    nc = _cache['nc']

    in_maps = []
    for c in range(NCORES):
        m = dict(Wd)
        m['xT'] = _pack_x(x[c*b:(c+1)*b])
        in_maps.append(m)
    res = run_bass_kernel_spmd(nc, in_maps, list(range(NCORES)))

    const = (np.maximum(g['dec_b1'], 0) @ g['dec_w2'] + g['dec_b2']).astype(np.float32)
    out = np.empty((B, NB, NP, 3), np.float32)
    out[:] = const
    for c in range(NCORES):
        o = np.asarray(res.results[c]['out'])      # [3, 9216]
        sl = o.T.reshape(NSLOT, b, 3).transpose(1, 0, 2)  # [sample, slot, 3]
        out[c*b:(c+1)*b, 0, :, :] = sl[:, :NP]
        out[c*b:(c+1)*b, 1, :CS[0], :] = sl[:, NP:NP+CS[0]]
        out[c*b:(c+1)*b, 2, :CS[1], :] = sl[:, NP+CS[0]:]
    return out


# revision 8
# speedup vs baseline: 1.4998x; 1.3486x over previous
"""Trainium2 Bass kernel for nn_BDLOTreeLSTM_78847009620604.

8-core data parallelism over batch (64 samples/core). Layout: [dim-partitions,
node-cols]; H=256 -> 2 partition halves; 144 (branch, vertex) slots x 64
samples = 9216 node columns per core (parent v -> slot v, c1 v -> 64+v,
c2 v -> 96+v).

- encoder L2 is folded into the pre-gate weights on host:
  pre = h1 @ (enc_w2 @ Wgate) + (enc_b2 @ Wgate + bgate),  h1 = relu(x@w1+b1)
- chain steps: pre-gates are accumulated into PSUM by windowed matmuls (bf16),
  the recurrent h @ U term accumulates on top; gate chunk order [i, u, f, o];
  sigmoid/tanh on ScalarE reading PSUM; cell update on VectorE (+ i*u on
  GPSIMD); h written in bf16 directly into the big state buffers.
- child chains c1/c2 run batched as one stream (slot cols [c1 | c2]).
- biases enter via rank-1 ones-row matmuls (K=1).
"""
import numpy as np
import ml_dtypes

B, H, NP, IN = 512, 256, 64, 9
CS = (32, 48)
COUP = (16, 40)
NB = 3
NCORES = 8
b = B // NCORES                  # 64
NSLOT = NP + CS[0] + CS[1]       # 144
NCOL = NSLOT * b                 # 9216
NG = NCOL // 512                 # 18
WIN = 2                          # chain pre-gate window (steps)

bf16 = ml_dtypes.bfloat16
_cache = {}


def _fuse_weights(g):
    def cat(*xs):
        return np.concatenate(xs, -1)
    Wbu = cat(g['bu_w_iou'][:, 2*H:3*H], g['bu_w_iou'][:, :H], g['bu_w_f'], g['bu_w_iou'][:, H:2*H])
    Ubu = cat(g['bu_u_iou'][:, 2*H:3*H], g['bu_u_iou'][:, :H], g['bu_u_f'], g['bu_u_iou'][:, H:2*H])
    bbu = cat(g['bu_b_iou'][2*H:3*H], g['bu_b_iou'][:H], g['bu_b_f'], g['bu_b_iou'][H:2*H])
    def td_re(w):
        return cat(w[..., 2*H:3*H], w[..., :H], w[..., H:2*H], w[..., 3*H:])
    Wtd = td_re(g['td_w_ih']); Utd = td_re(g['td_w_hh'])
    btd = td_re(g['td_b_ih'] + g['td_b_hh'])
    WpreBU = g['enc_w2'] @ Wbu; bpreBU = g['enc_b2'] @ Wbu + bbu
    WpreTD = g['enc_w2'] @ Wtd; bpreTD = g['enc_b2'] @ Wtd + btd

    def ktile(w):  # [256, C] -> [128, 2, C]
        return np.ascontiguousarray(w.reshape(2, 128, -1).transpose(1, 0, 2))

    return {
        'w1': np.ascontiguousarray(g['enc_w1']).astype(bf16),
        'b1t': np.ascontiguousarray(g['enc_b1'].reshape(2, 128).T).astype(np.float32),
        'WpreBU': ktile(WpreBU).astype(bf16), 'WpreTD': ktile(WpreTD).astype(bf16),
        'UBU': ktile(Ubu).astype(bf16), 'UTD': ktile(Utd).astype(bf16),
        'bpreBU': bpreBU.reshape(1, -1).astype(bf16), 'bpreTD': bpreTD.reshape(1, -1).astype(bf16),
        'dw1': np.ascontiguousarray(g['dec_w1'].reshape(4, 128, H).transpose(1, 0, 2)).astype(bf16),
        'db1': g['dec_b1'].reshape(1, -1).astype(bf16),
        'dw2': ktile(g['dec_w2']).astype(bf16),
        'db2': g['dec_b2'].reshape(1, -1).astype(bf16),
    }


def _pack_x(x_core):
    parts = [x_core[:, 0, :, :].transpose(1, 0, 2).reshape(NP * b, IN),
             x_core[:, 1, :CS[0], :].transpose(1, 0, 2).reshape(CS[0] * b, IN),
             x_core[:, 2, :CS[1], :].transpose(1, 0, 2).reshape(CS[1] * b, IN)]
    xs = np.concatenate(parts, 0)
    return np.ascontiguousarray(xs.T).astype(bf16)


def build_nc():
    import concourse.bass as bass
    import concourse.mybir as mybir
    import concourse.tile as tile
    from concourse import bacc
    from contextlib import ExitStack

    dt = mybir.dt
    AF = mybir.ActivationFunctionType
    AP = bass.AP

    nc = bacc.Bacc(None, target_bir_lowering=False)
    P = {}
    spec = [('xT', (IN, NCOL), dt.bfloat16), ('w1', (IN, H), dt.bfloat16),
            ('b1t', (128, 2), dt.float32),
            ('WpreBU', (128, 2, 4*H), dt.bfloat16), ('WpreTD', (128, 2, 4*H), dt.bfloat16),
            ('UBU', (128, 2, 4*H), dt.bfloat16), ('UTD', (128, 2, 4*H), dt.bfloat16),
            ('bpreBU', (1, 4*H), dt.bfloat16), ('bpreTD', (1, 4*H), dt.bfloat16),
            ('dw1', (128, 4, H), dt.bfloat16), ('db1', (1, H), dt.bfloat16),
            ('dw2', (128, 2, 3), dt.bfloat16), ('db2', (1, 3), dt.bfloat16)]
    for n, shape, d in spec:
        P[n] = nc.declare_dram_parameter(n, list(shape), d, isOutput=False)
    out_d = nc.declare_dram_parameter('out', [3, NCOL], dt.float32, isOutput=True)

    def full(handle, rank):
        return handle[tuple(slice(None) for _ in range(rank))]

    with tile.TileContext(nc) as tc, ExitStack() as ctx:
        sing = ctx.enter_context(tc.tile_pool(name="sing", bufs=1))
        xin = ctx.enter_context(tc.tile_pool(name="xin", bufs=3))
        gpool = ctx.enter_context(tc.tile_pool(name="gpool", bufs=4))
        tpool = ctx.enter_context(tc.tile_pool(name="tpool", bufs=4))
        cpool = ctx.enter_context(tc.tile_pool(name="cpool", bufs=3))
        opool = ctx.enter_context(tc.tile_pool(name="opool", bufs=3))

        W = {}
        for n, shape, d in spec:
            if n == 'xT':
                continue
            t = sing.tile(list(shape), d, tag=f"w_{n}")
            nc.sync.dma_start(out=t, in_=full(P[n], len(shape)))
            W[n] = t
        ones = sing.tile([1, 512], dt.bfloat16, tag="ones")
        nc.vector.memset(ones, 1.0)
        zrow = sing.tile([1, 1024], dt.bfloat16, tag="zrow")
        nc.vector.memset(zrow, 0.0)

        h1 = sing.tile([128, 2, NCOL], dt.bfloat16, tag="h1")
        bu_h = sing.tile([128, 2, NCOL], dt.bfloat16, tag="bu_h")
        td_h = sing.tile([128, 2, NCOL], dt.bfloat16, tag="td_h")
        croots = sing.tile([128, 2, 128], dt.float32, tag="croots")
        tdseed = sing.tile([128, 2, 128], dt.float32, tag="tdseed")

        # ---------------- encoder ----------------
        with tc.tile_pool(name="eps", bufs=2, space="PSUM") as eps:
            for gi in range(NG - 1, -1, -1):   # children cols first (phase A needs them)
                xg = xin.tile([IN, 512], dt.bfloat16, tag="xg")
                nc.sync.dma_start(out=xg, in_=P['xT'][:, gi*512:(gi+1)*512])
                pe = eps.tile([128, 2, 512], dt.float32, tag="pe")
                for t in range(2):
                    nc.tensor.matmul(pe[:, t, :], W['w1'][:, t*128:(t+1)*128], xg,
                                     start=True, stop=True)
                for t in range(2):
                    nc.scalar.activation(h1[:, t, gi*512:(gi+1)*512], pe[:, t, :],
                                         AF.Relu, bias=W['b1t'][:, t:t+1])

        # ---------------- helpers ----------------
        def hslice(buf, k, grp):
            """[128, 64] AP: buf[:, k, grp*b:(grp+1)*b]"""
            return buf[:, k, grp*b:(grp+1)*b]

        def hpair(buf, k, grp_lo, gstep):
            """[128, 2, 64] AP over groups {grp_lo, grp_lo+gstep}"""
            base = buf[:, k, :]
            return AP(tensor=base.tensor, offset=base.offset + grp_lo * b,
                      ap=[base.ap[0], [gstep * b, 2], [1, b]])

        def store_ap(buf, grp_lo, gstep, nbr):
            """[128, 2, nbr, 64] write AP into both hh halves of buf."""
            return AP(tensor=buf.tensor, offset=buf.offset + grp_lo * b,
                      ap=[buf.ap[0], [NCOL, 2], [gstep * b, nbr] if nbr > 1 else [0, 1],
                          [1, b]])

        def pre_window(ps, Wpre, bpre, rhs_fn, ncolw):
            cpb = 2048 // (ncolw * 4)       # chunks per PSUM bank
            for j in range(8):
                dst = ps[:, j, 0:ncolw]
                for k in range(2):
                    nc.tensor.matmul(dst, Wpre[:, k, j*128:(j+1)*128], rhs_fn(k),
                                     start=(k == 0 and j % cpb == 0), stop=False,
                                     skip_group_check=True)
                nc.tensor.matmul(dst, bpre[:, j*128:(j+1)*128], ones[:, 0:ncolw],
                                 start=False, stop=False, skip_group_check=True)

        def emit_step(ps, off, n, U, h_aps, c_prev, store, extra=None):
            """One LSTM step on psum cols [off, off+n). h_aps None => leaf step.
            c_prev: [128, 2, n] AP or None. store: [128, 2, nbr, 64] AP.
            extra: (eh_aps, ec_ap, fe_pool, pre_rhs) coupling extra child."""
            sl = lambda j: ps[:, j, off:off+n]
            if extra is not None:
                eh_aps = extra[0]
                for j in (0, 1, 2, 3, 6, 7):
                    for k in range(2):
                        nc.tensor.matmul(sl(j), U[:, k, j*128:(j+1)*128], eh_aps[k],
                                         start=False, stop=False, skip_group_check=True)
            if h_aps is not None:
                for j in range(8):
                    for k in range(2):
                        nc.tensor.matmul(sl(j), U[:, k, j*128:(j+1)*128], h_aps[k],
                                         start=False, stop=(k == 1), skip_group_check=True)
            else:
                for j in range(8):
                    nc.tensor.matmul(sl(j), zrow[:, j*128:(j+1)*128], ones[:, 0:n],
                                     start=False, stop=True, skip_group_check=True)
            gates = gpool.tile([128, 8, 128], dt.float32, tag="gates")
            g = lambda j0, j1: gates[:, j0:j1, 0:n]
            pin = lambda j0, j1: ps[:, j0:j1, off:off+n]
            nc.scalar.activation(g(0, 2), pin(0, 2), AF.Tanh)
            nc.scalar.activation(g(2, 8), pin(2, 8), AF.Sigmoid)
            t1 = tpool.tile([128, 2, 128], dt.float32, tag="t1")
            nc.gpsimd.tensor_mul(t1[:, :, 0:n], g(2, 4), g(0, 2))
            c_new = cpool.tile([128, 2, 128], dt.float32, tag="c")
            cn = c_new[:, :, 0:n]
            if h_aps is None and c_prev is None:
                nc.vector.tensor_copy(cn, t1[:, :, 0:n])
            else:
                nc.vector.tensor_mul(cn, g(4, 6), c_prev)
                nc.vector.tensor_add(cn, cn, t1[:, :, 0:n])
            if extra is not None:
                nc.vector.tensor_add(cn, cn, extra[1])
            tct = tpool.tile([128, 2, 128], dt.float32, tag="tc")
            nc.scalar.activation(tct[:, :, 0:n], cn, AF.Tanh)
            nbr = n // b
            nc.vector.tensor_mul(
                store,
                g(6, 8).rearrange("p c (r s) -> p c r s", s=b),
                tct[:, :, 0:n].rearrange("p c (r s) -> p c r s", s=b))
            return cn

        AluOp = mybir.AluOpType

        # ------------- phase A: bu children, v = 47..0, slots ascend v -------------
        with tc.tile_pool(name="pgA", bufs=2, space="PSUM") as pgA:
            c_prev = None
            ps = None
            for s in range(CS[1]):
                v = CS[1] - 1 - s
                if s % WIN == 0:
                    ps = pgA.tile([128, 8, WIN * 128], dt.float32, tag="pg")
                    v_lo = v - (WIN - 1)
                    def rhs(k, v_lo=v_lo):
                        bb = h1[:, k, :]
                        return AP(tensor=bb.tensor, offset=bb.offset + (64 + v_lo) * b,
                                  ap=[bb.ap[0], [b, WIN], [32 * b, 2], [1, b]])
                    pre_window(ps, W['WpreBU'], W['bpreBU'], rhs, WIN * 128)
                slot_i = (WIN - 1) - (s % WIN)          # ascending v in window
                off = slot_i * 128
                if v > CS[0] - 1:
                    # c2 only; c2 occupies cols [64:128) of the slot
                    h_aps = None if s == 0 else tuple(hslice(bu_h, k, 96 + v + 1) for k in range(2))
                    store = store_ap(bu_h, 96 + v, 0, 1)
                    c_prev = emit_step(ps, off + 64, 64, W['UBU'], h_aps, c_prev, store)
                elif v == CS[0] - 1:
                    # c1 leaf joins: c1 half (cols 0:64) first-step, c2 half normal
                    h_aps = tuple(hslice(bu_h, k, 96 + v + 1) for k in range(2))
                    sl = lambda j: ps[:, j, off+64:off+128]
                    for j in range(8):
                        for k in range(2):
                            nc.tensor.matmul(sl(j), W['UBU'][:, k, j*128:(j+1)*128],
                                             h_aps[k], start=False, stop=(k == 1),
                                             skip_group_check=True)
                    gates = gpool.tile([128, 8, 128], dt.float32, tag="gates")
                    pin = lambda j0, j1: ps[:, j0:j1, off:off+128]
                    nc.scalar.activation(gates[:, 0:2, :], pin(0, 2), AF.Tanh)
                    nc.scalar.activation(gates[:, 2:8, :], pin(2, 8), AF.Sigmoid)
                    t1 = tpool.tile([128, 2, 128], dt.float32, tag="t1")
                    nc.gpsimd.tensor_mul(t1, gates[:, 2:4, :], gates[:, 0:2, :])
                    c_new = cpool.tile([128, 2, 128], dt.float32, tag="c")
                    nc.vector.tensor_copy(c_new[:, :, 0:64], t1[:, :, 0:64])
                    nc.vector.tensor_mul(c_new[:, :, 64:128], gates[:, 4:6, 64:128], c_prev)
                    nc.vector.tensor_add(c_new[:, :, 64:128], c_new[:, :, 64:128],
                                         t1[:, :, 64:128])
                    tct = tpool.tile([128, 2, 128], dt.float32, tag="tc")
                    nc.scalar.activation(tct, c_new, AF.Tanh)
                    store = store_ap(bu_h, 64 + v, 32, 2)
                    nc.vector.tensor_mul(
                        store,
                        gates[:, 6:8, :].rearrange("p c (r s) -> p c r s", s=b),
                        tct.rearrange("p c (r s) -> p c r s", s=b))
                    c_prev = c_new
                else:
                    h_aps = tuple(hpair(bu_h, k, 64 + v + 1, 32) for k in range(2))
                    store = store_ap(bu_h, 64 + v, 32, 2)
                    c_prev = emit_step(ps, off, 128, W['UBU'], h_aps, c_prev, store)
            nc.vector.tensor_copy(croots, c_prev)

        # precompute coupling extra-child c contribution: sig(pre_f + eh@Uf) * ec
        cextra = sing.tile([128, 2, 128], dt.float32, tag="cextra")
        with tc.tile_pool(name="feP", bufs=2, space="PSUM") as feP:
            for idx, (vv, grp) in enumerate([(COUP[0], 64), (COUP[1], 96)]):
                fe = feP.tile([128, 2, b], dt.float32, tag="fe")
                eh = tuple(hslice(bu_h, k, grp) for k in range(2))
                prr = tuple(hslice(h1, k, vv) for k in range(2))
                for hh in range(2):
                    jj = 4 + hh
                    for k in range(2):
                        nc.tensor.matmul(fe[:, hh, :], W['WpreBU'][:, k, jj*128:(jj+1)*128],
                                         prr[k], start=(hh == 0 and k == 0), stop=False,
                                         skip_group_check=True)
                    nc.tensor.matmul(fe[:, hh, :], W['bpreBU'][:, jj*128:(jj+1)*128],
                                     ones[:, 0:b], start=False, stop=False,
                                     skip_group_check=True)
                    for k in range(2):
                        nc.tensor.matmul(fe[:, hh, :], W['UBU'][:, k, jj*128:(jj+1)*128],
                                         eh[k], start=False,
                                         stop=(hh == 1 and k == 1), skip_group_check=True)
                fes = tpool.tile([128, 2, b], dt.float32, tag="fes")
                nc.scalar.activation(fes, fe, AF.Sigmoid)
                nc.vector.tensor_mul(cextra[:, :, idx*64:(idx+1)*64], fes,
                                     croots[:, :, idx*64:(idx+1)*64])

        # ------------- phase B: bu parent, v = 63..0 -------------
        WP = 4
        with tc.tile_pool(name="pgB", bufs=2, space="PSUM") as pgB:
            c_prev = None
            ps = None
            for s in range(NP):
                v = NP - 1 - s
                if s % WP == 0:
                    ps = pgB.tile([128, 8, WP * 64], dt.float32, tag="pg")
                    v_lo = v - (WP - 1)
                    def rhs(k, v_lo=v_lo):
                        bb = h1[:, k, :]
                        return AP(tensor=bb.tensor, offset=bb.offset + v_lo * b,
                                  ap=[bb.ap[0], [b, WP], [1, b]])
                    pre_window(ps, W['WpreBU'], W['bpreBU'], rhs, WP * 64)
                off = ((WP - 1) - (s % WP)) * 64
                extra = None
                if v == COUP[0]:
                    eh = tuple(hslice(bu_h, k, 64) for k in range(2))
                    extra = (eh, cextra[:, :, 0:64])
                elif v == COUP[1]:
                    eh = tuple(hslice(bu_h, k, 96) for k in range(2))
                    extra = (eh, cextra[:, :, 64:128])
                h_aps = None if s == 0 else tuple(hslice(bu_h, k, v + 1) for k in range(2))
                store = store_ap(bu_h, v, 0, 1)
                c_prev = emit_step(ps, off, 64, W['UBU'], h_aps, c_prev, store, extra=extra)

        nc.vector.tensor_copy(td_h[:, :, 0:b], bu_h[:, :, 0:b])

        # ------------- phase C: td parent, v = 1..63 -------------
        with tc.tile_pool(name="pgC", bufs=2, space="PSUM") as pgC:
            WP = 4
            ps = None
            for i, v in enumerate(range(1, NP)):
                if i % WP == 0:
                    ps = pgC.tile([128, 8, WP * 64], dt.float32, tag="pg")
                    def rhs(k, v0=v):
                        bb = h1[:, k, :]
                        return AP(tensor=bb.tensor, offset=bb.offset + v0 * b,
                                  ap=[bb.ap[0], [b, WP], [1, b]])
                    pre_window(ps, W['WpreTD'], W['bpreTD'], rhs, WP * 64)
                off = (i % WP) * 64
                h_aps = tuple(hslice(td_h, k, v - 1) for k in range(2))
                store = store_ap(td_h, v, 0, 1)
                c_prev = emit_step(ps, off, 64, W['UTD'], h_aps, c_prev, store)
                if v == COUP[0]:
                    nc.vector.tensor_copy(tdseed[:, :, 0:64], c_prev)
                elif v == COUP[1]:
                    nc.vector.tensor_copy(tdseed[:, :, 64:128], c_prev)

        # ------------- phase D: td children, v = 0..47 -------------
        with tc.tile_pool(name="pgD", bufs=2, space="PSUM") as pgD:
            c_prev = None
            ps = None
            for s in range(CS[1]):
                v = s
                if s % WIN == 0:
                    ps = pgD.tile([128, 8, WIN * 128], dt.float32, tag="pg")
                    def rhs(k, v0=v):
                        bb = h1[:, k, :]
                        return AP(tensor=bb.tensor, offset=bb.offset + (64 + v0) * b,
                                  ap=[bb.ap[0], [b, WIN], [32 * b, 2], [1, b]])
                    pre_window(ps, W['WpreTD'], W['bpreTD'], rhs, WIN * 128)
                off = (s % WIN) * 128
                if s == 0:
                    h_aps = tuple(
                        AP(tensor=td_h[:, k, :].tensor,
                           offset=td_h[:, k, :].offset + COUP[0] * b,
                           ap=[td_h[:, k, :].ap[0], [(COUP[1] - COUP[0]) * b, 2], [1, b]])
                        for k in range(2))
                    store = store_ap(td_h, 64 + v, 32, 2)
                    c_prev = emit_step(ps, off, 128, W['UTD'], h_aps, tdseed, store)
                elif v < CS[0]:
                    h_aps = tuple(hpair(td_h, k, 64 + v - 1, 32) for k in range(2))
                    store = store_ap(td_h, 64 + v, 32, 2)
                    c_prev = emit_step(ps, off, 128, W['UTD'], h_aps, c_prev, store)
                else:
                    h_aps = tuple(hslice(td_h, k, 96 + v - 1) for k in range(2))
                    store = store_ap(td_h, 96 + v, 0, 1)
                    cp = c_prev[:, :, 64:128] if v == CS[0] else c_prev
                    c_prev = emit_step(ps, off + 64, 64, W['UTD'], h_aps, cp, store)

        # ------------- decoder -------------
        with tc.tile_pool(name="dps", bufs=2, space="PSUM") as dps, \
             tc.tile_pool(name="ops", bufs=2, space="PSUM") as ops:
            for gi in range(NG):
                cs0, cs1 = gi * 512, (gi + 1) * 512
                pd = dps.tile([128, 2, 512], dt.float32, tag="pd")
                for m in range(2):
                    for k in range(4):
                        srcbuf = bu_h if k < 2 else td_h
                        nc.tensor.matmul(pd[:, m, :], W['dw1'][:, k, m*128:(m+1)*128],
                                         srcbuf[:, k % 2, cs0:cs1],
                                         start=(k == 0), stop=False)
                    nc.tensor.matmul(pd[:, m, :], W['db1'][:, m*128:(m+1)*128],
                                     ones[:, 0:512], start=False, stop=True)
                h2 = opool.tile([128, 2, 512], dt.bfloat16, tag="h2")
                for m in range(2):
                    nc.scalar.activation(h2[:, m, :], pd[:, m, :], AF.Relu)
                po = ops.tile([3, 512], dt.float32, tag="po")
                for k in range(2):
                    nc.tensor.matmul(po, W['dw2'][:, k, :], h2[:, k, :],
                                     start=(k == 0), stop=False)
                nc.tensor.matmul(po, W['db2'], ones[:, 0:512], start=False, stop=True)
                og = opool.tile([3, 512], dt.float32, tag="og")
                nc.vector.tensor_copy(og, po)
                nc.sync.dma_start(out=out_d[:, cs0:cs1], in_=og)

    nc.finalize()
    return nc


def kernel(**inputs):
    from concourse.bass_utils import run_bass_kernel_spmd

    g = {k: np.asarray(v, np.float32) for k, v in inputs.items()}
    x = g['x']
    Wd = _fuse_weights(g)

    if 'nc' not in _cache:
        _cache['nc'] = build_nc()
    nc = _cache['nc']

    in_maps = []
    for c in range(NCORES):
        m = dict(Wd)
        m['xT'] = _pack_x(x[c*b:(c+1)*b])
        in_maps.append(m)
    res = run_bass_kernel_spmd(nc, in_maps, list(range(NCORES)))

    const = (np.maximum(g['dec_b1'], 0) @ g['dec_w2'] + g['dec_b2']).astype(np.float32)
    out = np.empty((B, NB, NP, 3), np.float32)
    out[:] = const
    for c in range(NCORES):
        o = np.asarray(res.results[c]['out'])      # [3, 9216]
        sl = o.T.reshape(NSLOT, b, 3).transpose(1, 0, 2)  # [sample, slot, 3]
        out[c*b:(c+1)*b, 0, :, :] = sl[:, :NP]
        out[c*b:(c+1)*b, 1, :CS[0], :] = sl[:, NP:NP+CS[0]]
        out[c*b:(c+1)*b, 2, :CS[1], :] = sl[:, NP+CS[0]:]
    return out
